# revision 9
# baseline (speedup 1.0000x reference)
"""Trainium2 Bass kernel for nn_DGL_Net (3-layer GraphConv GNN, 50000 nodes, 800k edges).

Strategy (8 NeuronCores, SPMD):
  - Host: relabel nodes into 392 balanced tiles of 128 nodes, 49 tiles per core.
    Per layer: local matmul (bf16) -> scale by c_src -> AllGather (4 row-chunks,
    overlapped with compute) -> per-edge dma_gather (4 SWDGE queues) -> one-hot
    (Sel) matmul aggregation in PSUM -> c_dst + bias (+relu / log_softmax).
  - HBM-locality gather schedule: within each tile, edges are sorted by src row
    and split into 16 equal "quantile" slices. Tiles are processed in groups of
    8 (all 8 PSUM agg accumulators live); each 1024-index gather call covers one
    (group, slice) block, so consecutive descriptors land ~0.8KB apart in the
    table (HBM row-buffer hits) instead of ~6KB.
  - Sel one-hot matrices are generated ON-CHIP (one DVE is_equal per call,
    iota tile vs broadcast dst lanes) instead of streaming 25.7MB/layer of
    precomputed one-hots from HBM.
  - Aggregation matmul is sel-stationary: PSUM[d,f] += Sel[e,d].T @ G[e,f] per
    128-edge chunk (one matmul per chunk). Per-tile epilogue applies c_dst
    (per-partition scalar) + bias + relu, PE-transposes back to [f,d] layout,
    and immediately runs the NEXT layer's dense matmul for that tile so the
    row-chunked AllGathers start early. Layer-3 epilogue computes log_softmax
    per tile (no serial tail).
  - The m*_full gather tables are laid out chunk-major ([all cores' seg-0 rows,
    then seg-1, ...]) so each chunked AllGather output is contiguous.
  - int16 gather indices: gather base is offset +32768 rows so idx = row-32768
    spans the whole [0, 50176) row space within int16. Every gather call ends
    in a light tile's slice tail pad (idx=0 >= 0, defeats the ucode's
    trailing-negative trim); pad slots carry dst=-1 so their Sel column is
    all-zero.
"""
import os
import sys

sys.path.insert(0, '/opt/trn_rl_repo')

import numpy as np
import ml_dtypes

import concourse.bass as bass
import concourse.bacc as bacc
import concourse.mybir as mybir
import concourse.tile as tile
from concourse.bass_utils import run_bass_kernel_spmd

BF16 = ml_dtypes.bfloat16

N_NODES = 50000
N_CORES = 8
TILE_N = 128                 # nodes per tile
TILES_PER_CORE = 49
N_TILES = N_CORES * TILES_PER_CORE      # 392
R_CHUNKS = 16                # src-quantile slices (chunks of 128 slots) per tile
SLOTS_PER_TILE = R_CHUNKS * 128          # 2048
SLOTS = TILES_PER_CORE * SLOTS_PER_TILE  # 100352 per core
ROWS_PER_CORE = TILES_PER_CORE * TILE_N  # 6272
N_ROWS = N_CORES * ROWS_PER_CORE         # 50176
CALL = 1024                  # idxs per dma_gather call
CPC = CALL // 128            # chunks per call (8)
N_CALLS = SLOTS // CALL      # 98
IDX_OFF = 32768              # gather base offset (int16 trick)
F_IN = 1433
F_IN_P = 1536                # padded to 12*128
KC1 = F_IN_P // 128          # 12
F1 = 256
F2 = 32
F3 = 7
FPAD = 128                   # padded row width for M2/M3 gather (256B elems)

# "light" tiles (cap 2032: every src-slice keeps a trailing pad so call-end
# slots always have idx>=0): last tile of each 8-group + tile 48
LIGHT_POS = {7, 15, 23, 31, 39, 47, 48}
CAP_HEAVY = 2048
CAP_LIGHT = R_CHUNKS * 127   # 2032
# AllGather row segments (tile counts); boundaries align with group ends
SEG_T = [16, 16, 8, 9]
SEG_R = [t * TILE_N for t in SEG_T]               # [2048, 2048, 1024, 1152]
SEG_START = [0, 2048, 4096, 5120]                 # per-core local row starts
SEG_FULL = [0, 16384, 32768, 40960]               # chunk-major full-table starts
AG_FIRE = {15: 0, 31: 1, 39: 2, 48: 3}            # t_idx -> segment to fire

last_exec_time_ns = None


def _preprocess(edge_index):
    """Graph preprocessing: normalization constants, node->(core,tile,lane)
    relabeling with balanced per-tile in-degree (per-tile caps: heavy 2048 /
    light 2032), c-major slot layout with per-tile src-sorted quantile slices."""
    src = np.asarray(edge_index[0], dtype=np.int64)
    dst = np.asarray(edge_index[1], dtype=np.int64)
    n_edges = src.shape[0]

    deg_out = np.bincount(src, minlength=N_NODES).astype(np.float64)
    deg_in = np.bincount(dst, minlength=N_NODES).astype(np.float64)
    c_src = (1.0 / np.sqrt(np.maximum(deg_out, 1.0))).astype(np.float32)
    c_dst = (1.0 / np.sqrt(np.maximum(deg_in, 1.0))).astype(np.float32)

    # --- greedy balanced tile packing by in-degree, with per-tile caps ---
    import heapq
    N_LIGHT = len(LIGHT_POS) * N_CORES           # 56
    cap = np.full(N_TILES, CAP_HEAVY, dtype=np.int64)
    cap[:N_LIGHT] = CAP_LIGHT                    # tiles 0..55 are light
    order = np.argsort(-deg_in, kind='stable')
    heap = [(0.0, 0, t) for t in range(N_TILES)]
    heapq.heapify(heap)
    tile_nodes = [[] for _ in range(N_TILES)]
    tile_load = np.zeros(N_TILES)
    deferred = []
    for v in order:
        dv = deg_in[v]
        while True:
            load, cnt, t = heapq.heappop(heap)
            if cnt >= TILE_N:
                continue  # stale/full
            if load + dv > cap[t]:
                deferred.append((load, cnt, t))
                continue
            break
        tile_nodes[t].append(int(v))
        tile_load[t] = load + dv
        heapq.heappush(heap, (load + dv, cnt + 1, t))
        for item in deferred:
            heapq.heappush(heap, item)
        deferred = []
    assert all(tile_load[t] <= cap[t] for t in range(N_TILES))

    # assign tiles to (core, k): light tiles -> LIGHT_POS, heavy -> the rest,
    # round-robin by load to balance cores
    light_ids = np.arange(N_LIGHT)
    heavy_ids = np.arange(N_LIGHT, N_TILES)
    lsort = light_ids[np.argsort(-tile_load[light_ids], kind='stable')]
    hsort = heavy_ids[np.argsort(-tile_load[heavy_ids], kind='stable')]
    light_pos = sorted(LIGHT_POS)
    heavy_pos = [k for k in range(TILES_PER_CORE) if k not in LIGHT_POS]
    tile_assign = np.empty((N_CORES, TILES_PER_CORE), dtype=np.int64)
    for i, k in enumerate(light_pos):
        for c in range(N_CORES):
            tile_assign[c, k] = lsort[i * N_CORES + c]
    for i, k in enumerate(heavy_pos):
        for c in range(N_CORES):
            tile_assign[c, k] = hsort[i * N_CORES + c]

    # row mapping: row = c*ROWS_PER_CORE + k*128 + lane
    row_of_node = np.full(N_NODES, -1, dtype=np.int64)
    node_of_row = np.full(N_ROWS, -1, dtype=np.int64)  # -1 = virtual pad node
    for c in range(N_CORES):
        for k in range(TILES_PER_CORE):
            t = tile_assign[c, k]
            base = c * ROWS_PER_CORE + k * TILE_N
            for lane, v in enumerate(tile_nodes[t]):
                row_of_node[v] = base + lane
                node_of_row[base + lane] = v
    assert (row_of_node >= 0).all()

    # gather-row renumbering (chunk-major full tables, 4 segments)
    seg_start = np.asarray(SEG_START)
    seg_r = np.asarray(SEG_R)
    seg_full = np.asarray(SEG_FULL)

    def grow_of_row(row):
        c = row // ROWS_PER_CORE
        r = row % ROWS_PER_CORE
        s = np.searchsorted(seg_start, r, side='right') - 1
        return seg_full[s] + c * seg_r[s] + (r - seg_start[s])

    # --- per-core edge slot tables (c-major layout) ---
    dst_row = row_of_node[dst]
    src_row = row_of_node[src]
    src_grow = grow_of_row(src_row)
    e_core = dst_row // ROWS_PER_CORE
    e_k = (dst_row % ROWS_PER_CORE) // TILE_N
    e_lane = dst_row % TILE_N

    idx_flat = np.zeros((N_CORES, SLOTS), dtype=np.int16)      # pad idx = 0
    dst_flat = np.full((N_CORES, SLOTS), -1, dtype=np.int16)   # pad dst = -1

    # sort edges by (core, k, src_grow); compute per-edge slice + position
    key = e_core * TILES_PER_CORE + e_k
    eorder = np.lexsort((src_grow, key))
    key_s = key[eorder]
    grp_start = np.searchsorted(key_s, np.arange(N_CORES * TILES_PER_CORE))
    grp_cnt = np.bincount(key_s, minlength=N_CORES * TILES_PER_CORE)
    p = np.arange(n_edges) - grp_start[key_s]        # sorted pos within tile
    m = grp_cnt[key_s]                               # tile's edge count
    base_sz = m // R_CHUNKS
    rem = m % R_CHUNKS
    cut = rem * (base_sz + 1)
    in_head = p < cut
    sl_c = np.where(in_head, p // np.maximum(base_sz + 1, 1),
                    rem + (p - cut) // np.maximum(base_sz, 1))
    q = np.where(in_head, p % np.maximum(base_sz + 1, 1),
                 (p - cut) % np.maximum(base_sz, 1))
    assert sl_c.max() < R_CHUNKS and q.max() < 128

    # slot offset of chunk (k, c) within a core's slot array
    kk = key_s % TILES_PER_CORE
    g_i = np.minimum(kk // 8, 5)                     # group index
    j_i = np.where(kk == 48, 0, kk % 8)              # tile pos within group
    blk = np.where(kk == 48, 128, 1024)              # (group, c) block size
    g_off = np.where(kk == 48, 6 * 16 * 1024, g_i * 16 * 1024)
    slot = g_off + sl_c * blk + j_i * 128 + q

    cores_s = key_s // TILES_PER_CORE
    idx_flat[cores_s, slot] = (src_grow[eorder] - IDX_OFF).astype(np.int16)
    dst_flat[cores_s, slot] = e_lane[eorder].astype(np.int16)

    # every gather call must end with idx >= 0 (ucode trims trailing negatives)
    assert (idx_flat[:, CALL - 1::CALL] >= 0).all()

    # wrap idx to [128, SLOTS/16] (idx i -> [i%16 replicated, i//16])
    cols = SLOTS // 16
    idx_tile = np.zeros((N_CORES, 128, cols), dtype=np.int16)
    for c in range(N_CORES):
        w = idx_flat[c].reshape(cols, 16).T  # [16, cols]
        idx_tile[c] = np.tile(w, (8, 1))

    # dst lane per slot, wrapped [128 lanes, n_chunks] bf16 (on-chip sel-gen)
    n_chunks = SLOTS // 128
    dstv = np.empty((N_CORES, 128, n_chunks), dtype=BF16)
    for c in range(N_CORES):
        dstv[c] = dst_flat[c].reshape(n_chunks, 128).T.astype(BF16)

    # per-core normalization tables
    cd_row = np.where(node_of_row >= 0, c_dst[np.maximum(node_of_row, 0)], 1.0)
    cs_row = np.where(node_of_row >= 0, c_src[np.maximum(node_of_row, 0)], 1.0)
    cd_core = cd_row.reshape(N_CORES, ROWS_PER_CORE).astype(np.float32)
    cs_core = cs_row.reshape(N_CORES, ROWS_PER_CORE).astype(np.float32)
    cdst_pp = cd_core.reshape(N_CORES, TILES_PER_CORE, 128).transpose(0, 2, 1).copy()
    csrc_t = cs_core.reshape(N_CORES, TILES_PER_CORE, 128).transpose(0, 2, 1).copy()

    return dict(row_of_node=row_of_node, node_of_row=node_of_row,
                idx_tile=idx_tile, dstv=dstv,
                cdst_pp=cdst_pp, csrc_t=csrc_t)


def _slot_to_chunk(slot0):
    """Map a 128-aligned slot offset to (t_idx, c)."""
    if slot0 < 6 * 16 * 1024:
        g = slot0 // 16384
        rem = slot0 % 16384
        return g * 8 + (rem % 1024) // 128, rem // 1024
    rem = slot0 - 6 * 16 * 1024
    return 48, rem // 128


def _build_nc():
    nc = bacc.Bacc("TRN2", target_bir_lowering=False, debug=False,
                   enable_asserts=True, num_devices=N_CORES, num_swdge_queues=4)
    dt = mybir.dt
    inp = {}
    inp['xT'] = nc.dram_tensor("xT", [F_IN_P, ROWS_PER_CORE], dt.bfloat16, kind="ExternalInput")
    inp['W1'] = nc.dram_tensor("W1", [F_IN_P, F1], dt.bfloat16, kind="ExternalInput")
    inp['W2'] = nc.dram_tensor("W2", [F1, F2], dt.bfloat16, kind="ExternalInput")
    inp['W3'] = nc.dram_tensor("W3", [F2, F3], dt.bfloat16, kind="ExternalInput")
    inp['idx'] = nc.dram_tensor("idx", [128, SLOTS // 16], dt.int16, kind="ExternalInput")
    inp['dstv'] = nc.dram_tensor("dstv", [128, SLOTS // 128], dt.bfloat16, kind="ExternalInput")
    inp['iota'] = nc.dram_tensor("iota", [128, CPC, 128], dt.bfloat16, kind="ExternalInput")
    inp['ident'] = nc.dram_tensor("ident", [128, 128], dt.bfloat16, kind="ExternalInput")
    inp['cdst_pp'] = nc.dram_tensor("cdst_pp", [128, TILES_PER_CORE], dt.float32, kind="ExternalInput")
    inp['csrc_t'] = nc.dram_tensor("csrc_t", [128, TILES_PER_CORE], dt.float32, kind="ExternalInput")
    inp['b1r'] = nc.dram_tensor("b1r", [128, F1], dt.float32, kind="ExternalInput")
    inp['b2r'] = nc.dram_tensor("b2r", [128, F2], dt.float32, kind="ExternalInput")
    inp['b3r'] = nc.dram_tensor("b3r", [128, F3], dt.float32, kind="ExternalInput")
    out_t = nc.dram_tensor("out", [ROWS_PER_CORE, F3], dt.float32, kind="ExternalOutput")

    m_own = {}
    for li, w in ((1, F1), (2, FPAD), (3, FPAD)):
        for s in range(4):
            m_own[li, s] = nc.dram_tensor(f"m{li}_own_{s}", [SEG_R[s], w], dt.bfloat16)
    m_full = {
        1: nc.dram_tensor("m1_full", [N_ROWS, F1], dt.bfloat16, addr_space="Shared"),
        2: nc.dram_tensor("m2_full", [N_ROWS, FPAD], dt.bfloat16, addr_space="Shared"),
        3: nc.dram_tensor("m3_full", [N_ROWS, FPAD], dt.bfloat16, addr_space="Shared"),
    }

    AL = mybir.AluOpType
    AF = mybir.ActivationFunctionType
    RG = [list(range(N_CORES))]

    def ag_seg(li, s):
        """AllGather segment s of layer li's table (contiguous chunk-major rows)."""
        lo = SEG_FULL[s]
        hi = lo + N_CORES * SEG_R[s]
        nc.gpsimd.collective_compute(
            "AllGather", AL.bypass, replica_groups=RG,
            ins=[m_own[li, s][:, :]], outs=[m_full[li][lo:hi, :]])

    def m_store(li, t_idx, ob, w):
        s = int(np.searchsorted(np.asarray(SEG_START), t_idx * 128, side='right')) - 1
        r = t_idx * 128 - SEG_START[s]
        nc.sync.dma_start(m_own[li, s][r:r + 128, 0:w], ob[:])
        if t_idx in AG_FIRE:
            ag_seg(li, AG_FIRE[t_idx])

    with tile.TileContext(nc) as tc:
        with tc.tile_pool(name="const", bufs=1) as constp, \
             tc.tile_pool(name="big", bufs=1) as bigp, \
             tc.tile_pool(name="xstream", bufs=2) as xp, \
             tc.tile_pool(name="work", bufs=4) as wp, \
             tc.tile_pool(name="gpool", bufs=4) as gp, \
             tc.tile_pool(name="selp", bufs=4) as selp, \
             tc.tile_pool(name="psA", bufs=4, space="PSUM") as psA, \
             tc.tile_pool(name="psT", bufs=2, space="PSUM") as psT, \
             tc.tile_pool(name="psmm", bufs=2, space="PSUM") as psmm:

            # ---- resident constants ----
            w1_t = constp.tile([128, KC1, F1], mybir.dt.bfloat16)
            nc.sync.dma_start(w1_t[:], inp['W1'].rearrange("(kc p) n -> p kc n", p=128))
            w2_t = constp.tile([128, 2, F2], mybir.dt.bfloat16)
            nc.sync.dma_start(w2_t[:], inp['W2'].rearrange("(kc p) n -> p kc n", p=128))
            w3_t = constp.tile([F2, F3], mybir.dt.bfloat16)
            nc.sync.dma_start(w3_t[:], inp['W3'][:, :])
            idx_t = constp.tile([128, SLOTS // 16], mybir.dt.int16)
            nc.sync.dma_start(idx_t[:], inp['idx'][:, :])
            dstv_t = constp.tile([128, SLOTS // 128], mybir.dt.bfloat16)
            nc.sync.dma_start(dstv_t[:], inp['dstv'][:, :])
            iota_t = constp.tile([128, CPC, 128], mybir.dt.bfloat16)
            nc.sync.dma_start(iota_t[:], inp['iota'][:, :, :])
            ident_t = constp.tile([128, 128], mybir.dt.bfloat16)
            nc.sync.dma_start(ident_t[:], inp['ident'][:, :])
            cdpp_t = constp.tile([128, TILES_PER_CORE], mybir.dt.float32)
            nc.sync.dma_start(cdpp_t[:], inp['cdst_pp'][:, :])
            cs_t = constp.tile([128, TILES_PER_CORE], mybir.dt.float32)
            nc.sync.dma_start(cs_t[:], inp['csrc_t'][:, :])
            b1r_t = constp.tile([128, F1], mybir.dt.float32)
            nc.sync.dma_start(b1r_t[:], inp['b1r'][:, :])
            b2r_t = constp.tile([128, F2], mybir.dt.float32)
            nc.sync.dma_start(b2r_t[:], inp['b2r'][:, :])
            b3r_t = constp.tile([128, F3], mybir.dt.float32)
            nc.sync.dma_start(b3r_t[:], inp['b3r'][:, :])

            h1t = bigp.tile([128, 2, ROWS_PER_CORE], mybir.dt.bfloat16)  # H1.T
            h2t = bigp.tile([F2, ROWS_PER_CORE], mybir.dt.bfloat16)      # H2.T

            # ---- phase 1: M1 = (X @ W1) * c_src; AllGather per segment ----
            blocks = [(i * 512, 512) for i in range(12)] + [(6144, 128)]
            for c0, bs in blocks:
                xt = xp.tile([128, KC1, bs], mybir.dt.bfloat16, tag="xt")
                nc.sync.dma_start(
                    xt[:, :, :bs],
                    inp['xT'][:, c0:c0 + bs].rearrange("(kc p) n -> p kc n", p=128))
                for sub in range(bs // 128):
                    t_idx = (c0 + sub * 128) // 128
                    ps = psmm.tile([128, F1], mybir.dt.float32, tag="mm1", name="ps1")
                    for kc in range(KC1):
                        nc.tensor.matmul(ps[:], xt[:, kc, sub * 128:(sub + 1) * 128],
                                         w1_t[:, kc, :], start=(kc == 0), stop=(kc == KC1 - 1))
                    ob = wp.tile([128, F1], mybir.dt.bfloat16, tag="m1o")
                    nc.vector.tensor_scalar(ob[:], ps[:], cs_t[:, t_idx:t_idx + 1], None, AL.mult)
                    m_store(1, t_idx, ob, F1)

            # ---- agg: gather + on-chip sel + sel-stationary matmul, c-major ----
            def agg_layer(li, elem, fwidth, finish_tile):
                pairs = {}   # two agg accumulators share one PSUM bank
                for call in range(N_CALLS):
                    g = gp.tile([128, CPC, elem], mybir.dt.bfloat16, tag=f"g{elem}")
                    nc.gpsimd.dma_gather(
                        g[:], m_full[li][IDX_OFF:, :],
                        idx_t[:, call * (CALL // 16):(call + 1) * (CALL // 16)],
                        CALL, CALL, elem, queue_num=call % 4)
                    selg = selp.tile([128, CPC, 128], mybir.dt.bfloat16, tag="selg")
                    ch0 = call * CPC
                    nc.vector.tensor_tensor(
                        selg[:], iota_t[:],
                        dstv_t[:, ch0:ch0 + CPC].unsqueeze(2).broadcast_to([128, CPC, 128]),
                        AL.is_equal)
                    for j in range(CPC):
                        t_idx, c = _slot_to_chunk((ch0 + j) * 128)
                        first = (c == 0)
                        last = (c == R_CHUNKS - 1)
                        if first and t_idx // 2 not in pairs:
                            pairs[t_idx // 2] = psA.tile([128, 2, fwidth], mybir.dt.float32,
                                                         tag="aggA", name="psagg")
                        acc = pairs[t_idx // 2][:, t_idx % 2, :]
                        nc.tensor.matmul(acc, selg[:, j, :], g[:, j, 0:fwidth],
                                         start=first, stop=last)
                        if last:
                            finish_tile(acc, t_idx)
                            if t_idx % 2 == 1 or t_idx == 48:
                                pairs.pop(t_idx // 2)

            # ---- layer 1 epilogue: H1 tile + fused L2 dense + AG2 ----
            def l1_tile(ps_agg, t_idx):
                sl = slice(t_idx * 128, (t_idx + 1) * 128)
                h1d = wp.tile([128, F1], mybir.dt.bfloat16, tag="h1d")
                nc.vector.scalar_tensor_tensor(
                    h1d[:], ps_agg[:], cdpp_t[:, t_idx:t_idx + 1], b1r_t[:],
                    AL.mult, AL.add)
                nc.scalar.activation(h1d[:], h1d[:], AF.Relu)
                for fc in range(2):
                    trp = psT.tile([128, 128], mybir.dt.bfloat16, tag="tr", name="trp")
                    nc.tensor.transpose(trp[:], h1d[:, fc * 128:(fc + 1) * 128], ident_t[:])
                    nc.scalar.activation(h1t[:, fc, sl], trp[:], AF.Copy)
                ps2 = psmm.tile([128, F1], mybir.dt.float32, tag="mm1", name="ps2")
                for fc in range(2):
                    nc.tensor.matmul(ps2[:, 0:F2], h1t[:, fc, sl], w2_t[:, fc, :],
                                     start=(fc == 0), stop=(fc == 1))
                ob2 = wp.tile([128, F2], mybir.dt.bfloat16, tag="ob2")
                nc.vector.tensor_scalar(ob2[:], ps2[:, 0:F2], cs_t[:, t_idx:t_idx + 1], None, AL.mult)
                m_store(2, t_idx, ob2, F2)

            agg_layer(1, F1, F1, l1_tile)

            # ---- layer 2 epilogue: H2 tile + fused L3 dense + AG3 ----
            def l2_tile(ps_agg, t_idx):
                sl = slice(t_idx * 128, (t_idx + 1) * 128)
                h2d = wp.tile([128, F2], mybir.dt.bfloat16, tag="h2d")
                nc.vector.scalar_tensor_tensor(
                    h2d[:], ps_agg[:], cdpp_t[:, t_idx:t_idx + 1], b2r_t[:],
                    AL.mult, AL.add)
                nc.scalar.activation(h2d[:], h2d[:], AF.Relu)
                trp = psT.tile([128, 128], mybir.dt.bfloat16, tag="tr", name="trp2")
                nc.tensor.transpose(trp[0:F2, :], h2d[:], ident_t[:])
                nc.scalar.activation(h2t[:, sl], trp[0:F2, :], AF.Copy)
                ps3 = psmm.tile([128, F1], mybir.dt.float32, tag="mm1", name="ps3")
                nc.tensor.matmul(ps3[:, 0:F3], h2t[:, sl], w3_t[:], start=True, stop=True)
                ob3 = wp.tile([128, F3], mybir.dt.bfloat16, tag="ob3")
                nc.vector.tensor_scalar(ob3[:], ps3[:, 0:F3], cs_t[:, t_idx:t_idx + 1], None, AL.mult)
                m_store(3, t_idx, ob3, F3)

            agg_layer(2, FPAD, F2, l2_tile)

            # ---- layer 3 epilogue: per-tile log_softmax -> out ----
            def l3_tile(ps_agg, t_idx):
                xs = wp.tile([128, F3], mybir.dt.float32, tag="xs")
                nc.vector.scalar_tensor_tensor(
                    xs[:], ps_agg[:], cdpp_t[:, t_idx:t_idx + 1], b3r_t[:],
                    AL.mult, AL.add)
                ex = wp.tile([128, F3], mybir.dt.float32, tag="ex")
                nc.scalar.activation(ex[:], xs[:], AF.Exp)
                sm = wp.tile([128, 1], mybir.dt.float32, tag="sm")
                nc.vector.tensor_reduce(sm[:], ex[:], mybir.AxisListType.X, AL.add)
                rs = wp.tile([128, 1], mybir.dt.float32, tag="rs")
                nc.vector.reciprocal(rs[:], sm[:])
                ln = wp.tile([128, 1], mybir.dt.float32, tag="ln")
                nc.scalar.activation(ln[:], rs[:], AF.Ln)
                ox = wp.tile([128, F3], mybir.dt.float32, tag="ox")
                nc.vector.tensor_scalar(ox[:], xs[:], ln[:, 0:1], None, AL.add)
                nc.sync.dma_start(out_t[t_idx * 128:(t_idx + 1) * 128, :], ox[:])

            agg_layer(3, FPAD, F3, l3_tile)

    nc.compile()
    return nc


def _install_profile_shim():
    """Provide the missing antenv.axon_hooks module so trace=True works under axon."""
    try:
        import types
        import antenv
        if 'antenv.axon_hooks' in sys.modules:
            return
        _hook = [None]
        mod = types.ModuleType('antenv.axon_hooks')
        mod.set_axon_ntff_profile_hook = lambda h: _hook.__setitem__(0, h)
        mod.get_axon_ntff_profile_hook = lambda: _hook[0]
        sys.modules['antenv.axon_hooks'] = mod
        antenv.axon_hooks = mod
        from trn_agent_boot.trn_boot import _ntff_profile_via_ctypes
        mod.set_axon_ntff_profile_hook(
            _ntff_profile_via_ctypes('/opt/axon/libaxon_pjrt.so'))
    except Exception:
        pass


_CACHE = {}


def kernel(features, edge_index, W1, b1, W2, b2, W3, b3):
    global last_exec_time_ns
    features = np.asarray(features, dtype=np.float32)
    pre = _preprocess(np.asarray(edge_index))

    if 'nc' not in _CACHE:
        _CACHE['nc'] = _build_nc()
    nc = _CACHE['nc']

    # host-side input prep
    W1p = np.zeros((F_IN_P, F1), dtype=BF16)
    W1p[:F_IN] = np.asarray(W1, dtype=BF16)
    W2b = np.asarray(W2, dtype=BF16)
    W3b = np.asarray(W3, dtype=BF16)
    b1r = np.tile(np.asarray(b1, dtype=np.float32), (128, 1))
    b2r = np.tile(np.asarray(b2, dtype=np.float32), (128, 1))
    b3r = np.tile(np.asarray(b3, dtype=np.float32), (128, 1))
    iota = np.ascontiguousarray(np.broadcast_to(
        np.arange(128, dtype=np.float32), (128, CPC, 128))).astype(BF16)
    ident = np.eye(128, dtype=BF16)

    # features, permuted and transposed per core: [F_IN_P, 6272] bf16
    feat_b = features.astype(BF16)
    in_maps = []
    for c in range(N_CORES):
        rows = pre['node_of_row'][c * ROWS_PER_CORE:(c + 1) * ROWS_PER_CORE]
        xTc = np.zeros((F_IN_P, ROWS_PER_CORE), dtype=BF16)
        real = rows >= 0
        xTc[:F_IN, real] = feat_b[rows[real]].T
        in_maps.append({
            'xT': xTc, 'W1': W1p, 'W2': W2b, 'W3': W3b,
            'idx': pre['idx_tile'][c], 'dstv': pre['dstv'][c],
            'iota': iota, 'ident': ident,
            'cdst_pp': pre['cdst_pp'][c], 'csrc_t': pre['csrc_t'][c],
            'b1r': b1r, 'b2r': b2r, 'b3r': b3r,
        })

    trace = os.environ.get('BASS_KERNEL_TRACE', '0') == '1'
    if trace:
        _install_profile_shim()
    res = run_bass_kernel_spmd(nc, in_maps, core_ids=list(range(N_CORES)), trace=trace)
    last_exec_time_ns = res.exec_time_ns

    # assemble + inverse permute
    out_rows = np.concatenate([res.results[c]['out'] for c in range(N_CORES)], axis=0)
    out = np.empty((N_NODES, F3), dtype=np.float32)
    real = pre['node_of_row'] >= 0
    out[pre['node_of_row'][real]] = out_rows[real]
    return out


# revision 11
# speedup vs baseline: 1.3172x; 1.3172x over previous
"""Trainium2 Bass kernel for nn_DGL_Net (3-layer GraphConv GNN, 50000 nodes, 800k edges).

Strategy (8 NeuronCores, SPMD):
  - Host: relabel nodes into 392 balanced tiles of 128 nodes (<=2046 in-edges per
    tile), 49 tiles per core. Per layer: local matmul (bf16) -> scale by c_src ->
    AllGather (row-chunked, overlapped) -> per-edge dma_gather (4 SWDGE queues,
    src-sorted within each tile for HBM locality) -> one-hot (Sel) matmul
    aggregation in PSUM -> scale by c_dst + bias (+relu / log_softmax).
  - Sel one-hot matrices are generated ON-CHIP (one DVE is_equal per 1024-edge
    gather call, comparing a resident iota tile against broadcast dst lanes)
    instead of streaming 25.7MB/layer of precomputed one-hots from HBM.
  - Aggregation matmul is sel-stationary: PSUM[d,f] += Sel[e,d].T @ G[e,f] per
    128-edge chunk (one matmul per chunk). Per-tile epilogue applies
    c_dst (per-partition scalar) + bias + relu, PE-transposes back to [f,d]
    layout, and immediately runs the NEXT layer's dense matmul for that tile
    so the AllGathers can start early (chunked, overlapped with compute).
  - int16 gather indices: gather base is offset +32768 rows so idx = row-32768
    spans the whole [0, 50176) row space within int16. The last slot of every
    1024-index gather call is a reserved dummy with idx>=0 (defeats the ucode's
    trailing-negative trim).
"""
import os
import sys

sys.path.insert(0, '/opt/trn_rl_repo')

import numpy as np
import ml_dtypes

import concourse.bass as bass
import concourse.bacc as bacc
import concourse.mybir as mybir
import concourse.tile as tile
from concourse.bass_utils import run_bass_kernel_spmd

BF16 = ml_dtypes.bfloat16

N_NODES = 50000
N_CORES = 8
TILE_N = 128                 # nodes per tile
TILES_PER_CORE = 49
N_TILES = N_CORES * TILES_PER_CORE      # 392
ROWS_PER_CORE = TILES_PER_CORE * TILE_N  # 6272
N_ROWS = N_CORES * ROWS_PER_CORE         # 50176
R_CHUNKS = 16                # edge chunks (of 128 slots) per tile
SLOTS_PER_TILE = R_CHUNKS * 128          # 2048
TILE_EDGE_CAP = SLOTS_PER_TILE - 2       # 2046 (2 reserved call-end dummies)
SLOTS = TILES_PER_CORE * SLOTS_PER_TILE  # 100352 per core
CALL = 1024                  # idxs per dma_gather call
CPC = CALL // 128            # chunks per call (8)
N_CALLS = SLOTS // CALL      # 98
CHUNKS = TILES_PER_CORE * R_CHUNKS       # 784 chunks per core
IDX_OFF = 32768              # gather base offset (int16 trick)
F_IN = 1433
F_IN_P = 1536                # padded to 12*128
KC1 = F_IN_P // 128          # 12
F1 = 256
F2 = 32
F3 = 7
FPAD = 128                   # padded row width for M2/M3 gather (256B elems)
# AllGather row segments (tile counts); fired as each segment's tiles finish
SEG_T = [16, 16, 8, 9]
SEG_R = [t * TILE_N for t in SEG_T]               # [2048, 2048, 1024, 1152]
SEG_START = [0, 2048, 4096, 5120]                 # per-core local row starts
SEG_FULL = [0, 16384, 32768, 40960]               # chunk-major full-table starts
AG_FIRE = {15: 0, 31: 1, 39: 2, 48: 3}            # t_idx -> segment to fire

last_exec_time_ns = None


def _preprocess(edge_index):
    """Graph preprocessing: normalization constants, node->($core,tile,lane)
    relabeling with balanced per-tile in-degree, per-core edge slot tables
    (slots sorted by src row within each tile for gather locality)."""
    src = np.asarray(edge_index[0], dtype=np.int64)
    dst = np.asarray(edge_index[1], dtype=np.int64)
    n_edges = src.shape[0]

    deg_out = np.bincount(src, minlength=N_NODES).astype(np.float64)
    deg_in = np.bincount(dst, minlength=N_NODES).astype(np.float64)
    c_src = (1.0 / np.sqrt(np.maximum(deg_out, 1.0))).astype(np.float32)
    c_dst = (1.0 / np.sqrt(np.maximum(deg_in, 1.0))).astype(np.float32)

    # --- greedy balanced tile packing by in-degree ---
    import heapq
    order = np.argsort(-deg_in, kind='stable')
    heap = [(0.0, 0, t) for t in range(N_TILES)]  # (load, count, tile)
    heapq.heapify(heap)
    tile_nodes = [[] for _ in range(N_TILES)]
    tile_load = np.zeros(N_TILES)
    deferred = []
    for v in order:
        dv = deg_in[v]
        while True:
            load, cnt, t = heapq.heappop(heap)
            if cnt >= TILE_N:
                continue  # stale/full
            if load + dv > TILE_EDGE_CAP:
                deferred.append((load, cnt, t))
                continue
            break
        tile_nodes[t].append(int(v))
        tile_load[t] = load + dv
        heapq.heappush(heap, (load + dv, cnt + 1, t))
        for item in deferred:
            heapq.heappush(heap, item)
        deferred = []
    assert max(tile_load) <= TILE_EDGE_CAP

    # sort tiles by load desc, group by 8, core c takes c-th of each group
    tsort = np.argsort(-tile_load, kind='stable')
    tile_assign = np.empty((N_CORES, TILES_PER_CORE), dtype=np.int64)
    for k in range(TILES_PER_CORE):
        for c in range(N_CORES):
            tile_assign[c, k] = tsort[k * N_CORES + c]

    # row mapping: row = c*ROWS_PER_CORE + k*128 + lane
    row_of_node = np.full(N_NODES, -1, dtype=np.int64)
    node_of_row = np.full(N_ROWS, -1, dtype=np.int64)  # -1 = virtual pad node
    for c in range(N_CORES):
        for k in range(TILES_PER_CORE):
            t = tile_assign[c, k]
            nodes = tile_nodes[t]
            base = c * ROWS_PER_CORE + k * TILE_N
            for lane, v in enumerate(nodes):
                row_of_node[v] = base + lane
                node_of_row[base + lane] = v
    assert (row_of_node >= 0).all()

    # --- per-core edge slot tables ---
    dst_row = row_of_node[dst]
    src_row = row_of_node[src]
    e_core = dst_row // ROWS_PER_CORE
    e_tile = (dst_row % ROWS_PER_CORE) // TILE_N   # k within core
    e_lane = dst_row % TILE_N

    idx_flat = np.zeros((N_CORES, SLOTS), dtype=np.int16)      # pad idx = 0
    dst_flat = np.full((N_CORES, SLOTS), -1, dtype=np.int16)   # pad dst = -1

    # gather-row renumbering: full tables are laid out chunk-major
    # ([all cores' seg-0 rows, then seg-1, ...]) so each chunked AllGather
    # output is contiguous
    seg_start = np.asarray(SEG_START)
    seg_r = np.asarray(SEG_R)
    seg_full = np.asarray(SEG_FULL)
    sc = src_row // ROWS_PER_CORE
    sr = src_row % ROWS_PER_CORE
    seg_i = np.searchsorted(seg_start, sr, side='right') - 1
    src_grow = seg_full[seg_i] + sc * seg_r[seg_i] + (sr - seg_start[seg_i])

    # group edges by (core, tile); within each tile sort by src row (gather
    # locality), then assign slot positions skipping reserved slots 1023/2047
    key = e_core * TILES_PER_CORE + e_tile
    eorder = np.lexsort((src_grow, key))   # sort by key, then src gather-row
    key_s = key[eorder]
    grp_start = np.searchsorted(key_s, np.arange(N_CORES * TILES_PER_CORE))
    pos_in_grp = np.arange(n_edges) - grp_start[key_s]
    j = pos_in_grp
    slot_in_tile = j + (j >= 1023).astype(np.int64)  # j>=1023 shifts past slot 1023
    assert slot_in_tile.max() < SLOTS_PER_TILE - 1   # never hits 2047
    slots_abs = key_s % TILES_PER_CORE * SLOTS_PER_TILE + slot_in_tile
    cores_s = key_s // TILES_PER_CORE
    idx_flat[cores_s, slots_abs] = (src_grow[eorder] - IDX_OFF).astype(np.int16)
    dst_flat[cores_s, slots_abs] = e_lane[eorder].astype(np.int16)

    # wrap idx to [128, SLOTS/16] (idx i -> [i%16 replicated, i//16])
    cols = SLOTS // 16
    idx_tile = np.zeros((N_CORES, 128, cols), dtype=np.int16)
    for c in range(N_CORES):
        w = idx_flat[c].reshape(cols, 16).T  # [16, cols]
        idx_tile[c] = np.tile(w, (8, 1))

    # dst lane per slot, wrapped [128 lanes, CHUNKS] bf16 (for on-chip sel-gen)
    dstv = np.empty((N_CORES, 128, CHUNKS), dtype=BF16)
    for c in range(N_CORES):
        dstv[c] = dst_flat[c].reshape(CHUNKS, 128).T.astype(BF16)

    # per-core normalization tables
    cd_row = np.where(node_of_row >= 0, c_dst[np.maximum(node_of_row, 0)], 1.0)
    cs_row = np.where(node_of_row >= 0, c_src[np.maximum(node_of_row, 0)], 1.0)
    cd_core = cd_row.reshape(N_CORES, ROWS_PER_CORE).astype(np.float32)
    cs_core = cs_row.reshape(N_CORES, ROWS_PER_CORE).astype(np.float32)
    cdst_pp = cd_core.reshape(N_CORES, TILES_PER_CORE, 128).transpose(0, 2, 1).copy()
    csrc_t = cs_core.reshape(N_CORES, TILES_PER_CORE, 128).transpose(0, 2, 1).copy()

    return dict(row_of_node=row_of_node, node_of_row=node_of_row,
                idx_tile=idx_tile, dstv=dstv,
                cdst_pp=cdst_pp, csrc_t=csrc_t)


def _build_nc():
    nc = bacc.Bacc("TRN2", target_bir_lowering=False, debug=False,
                   enable_asserts=True, num_devices=N_CORES, num_swdge_queues=4)
    dt = mybir.dt
    inp = {}
    inp['xT'] = nc.dram_tensor("xT", [F_IN_P, ROWS_PER_CORE], dt.bfloat16, kind="ExternalInput")
    inp['W1'] = nc.dram_tensor("W1", [F_IN_P, F1], dt.bfloat16, kind="ExternalInput")
    inp['W2'] = nc.dram_tensor("W2", [F1, F2], dt.bfloat16, kind="ExternalInput")
    inp['W3'] = nc.dram_tensor("W3", [F2, F3], dt.bfloat16, kind="ExternalInput")
    inp['idx'] = nc.dram_tensor("idx", [128, SLOTS // 16], dt.int16, kind="ExternalInput")
    inp['dstv'] = nc.dram_tensor("dstv", [128, CHUNKS], dt.bfloat16, kind="ExternalInput")
    inp['iota'] = nc.dram_tensor("iota", [128, CPC, 128], dt.bfloat16, kind="ExternalInput")
    inp['ident'] = nc.dram_tensor("ident", [128, 128], dt.bfloat16, kind="ExternalInput")
    inp['cdst_pp'] = nc.dram_tensor("cdst_pp", [128, TILES_PER_CORE], dt.float32, kind="ExternalInput")
    inp['csrc_t'] = nc.dram_tensor("csrc_t", [128, TILES_PER_CORE], dt.float32, kind="ExternalInput")
    inp['b1r'] = nc.dram_tensor("b1r", [128, F1], dt.float32, kind="ExternalInput")
    inp['b2r'] = nc.dram_tensor("b2r", [128, F2], dt.float32, kind="ExternalInput")
    inp['b3r'] = nc.dram_tensor("b3r", [128, F3], dt.float32, kind="ExternalInput")
    out_t = nc.dram_tensor("out", [ROWS_PER_CORE, F3], dt.float32, kind="ExternalOutput")

    m_own = {}
    for li, w in ((1, F1), (2, FPAD), (3, FPAD)):
        for sg in range(4):
            m_own[li, sg] = nc.dram_tensor(f"m{li}_own_{sg}", [SEG_R[sg], w], dt.bfloat16)
    m_full = {
        1: nc.dram_tensor("m1_full", [N_ROWS, F1], dt.bfloat16, addr_space="Shared"),
        2: nc.dram_tensor("m2_full", [N_ROWS, FPAD], dt.bfloat16, addr_space="Shared"),
        3: nc.dram_tensor("m3_full", [N_ROWS, FPAD], dt.bfloat16, addr_space="Shared"),
    }

    AL = mybir.AluOpType
    AF = mybir.ActivationFunctionType
    RG = [list(range(N_CORES))]

    def ag_seg(li, sg):
        """AllGather segment sg of layer li's table (contiguous chunk-major rows)."""
        lo = SEG_FULL[sg]
        hi = lo + N_CORES * SEG_R[sg]
        nc.gpsimd.collective_compute(
            "AllGather", AL.bypass, replica_groups=RG,
            ins=[m_own[li, sg][:, :]], outs=[m_full[li][lo:hi, :]])

    def m_store(li, t_idx, ob, w):
        sg = int(np.searchsorted(np.asarray(SEG_START), t_idx * 128, side='right')) - 1
        r = t_idx * 128 - SEG_START[sg]
        nc.sync.dma_start(m_own[li, sg][r:r + 128, 0:w], ob[:])
        if t_idx in AG_FIRE:
            ag_seg(li, AG_FIRE[t_idx])

    with tile.TileContext(nc) as tc:
        with tc.tile_pool(name="const", bufs=1) as constp, \
             tc.tile_pool(name="big", bufs=1) as bigp, \
             tc.tile_pool(name="xstream", bufs=2) as xp, \
             tc.tile_pool(name="work", bufs=4) as wp, \
             tc.tile_pool(name="gpool", bufs=4) as gp, \
             tc.tile_pool(name="selp", bufs=4) as selp, \
             tc.tile_pool(name="psA", bufs=2, space="PSUM") as psA, \
             tc.tile_pool(name="psT", bufs=2, space="PSUM") as psT, \
             tc.tile_pool(name="psmm", bufs=2, space="PSUM") as psmm:

            # ---- resident constants ----
            w1_t = constp.tile([128, KC1, F1], mybir.dt.bfloat16)
            nc.sync.dma_start(w1_t[:], inp['W1'].rearrange("(kc p) n -> p kc n", p=128))
            w2_t = constp.tile([128, 2, F2], mybir.dt.bfloat16)
            nc.sync.dma_start(w2_t[:], inp['W2'].rearrange("(kc p) n -> p kc n", p=128))
            w3_t = constp.tile([F2, F3], mybir.dt.bfloat16)
            nc.sync.dma_start(w3_t[:], inp['W3'][:, :])
            idx_t = constp.tile([128, SLOTS // 16], mybir.dt.int16)
            nc.sync.dma_start(idx_t[:], inp['idx'][:, :])
            dstv_t = constp.tile([128, CHUNKS], mybir.dt.bfloat16)
            nc.sync.dma_start(dstv_t[:], inp['dstv'][:, :])
            iota_t = constp.tile([128, CPC, 128], mybir.dt.bfloat16)
            nc.sync.dma_start(iota_t[:], inp['iota'][:, :, :])
            ident_t = constp.tile([128, 128], mybir.dt.bfloat16)
            nc.sync.dma_start(ident_t[:], inp['ident'][:, :])
            cdpp_t = constp.tile([128, TILES_PER_CORE], mybir.dt.float32)
            nc.sync.dma_start(cdpp_t[:], inp['cdst_pp'][:, :])
            cs_t = constp.tile([128, TILES_PER_CORE], mybir.dt.float32)
            nc.sync.dma_start(cs_t[:], inp['csrc_t'][:, :])
            b1r_t = constp.tile([128, F1], mybir.dt.float32)
            nc.sync.dma_start(b1r_t[:], inp['b1r'][:, :])
            b2r_t = constp.tile([128, F2], mybir.dt.float32)
            nc.sync.dma_start(b2r_t[:], inp['b2r'][:, :])
            b3r_t = constp.tile([128, F3], mybir.dt.float32)
            nc.sync.dma_start(b3r_t[:], inp['b3r'][:, :])

            h1t = bigp.tile([128, 2, ROWS_PER_CORE], mybir.dt.bfloat16)  # H1.T
            h2t = bigp.tile([F2, ROWS_PER_CORE], mybir.dt.bfloat16)      # H2.T

            # ---- phase 1: M1 = (X @ W1) * c_src, row-chunked AllGather ----
            blocks = [(i * 512, 512) for i in range(12)] + [(6144, 128)]
            for c0, bs in blocks:
                xt = xp.tile([128, KC1, bs], mybir.dt.bfloat16, tag="xt")
                nc.sync.dma_start(
                    xt[:, :, :bs],
                    inp['xT'][:, c0:c0 + bs].rearrange("(kc p) n -> p kc n", p=128))
                for sub in range(bs // 128):
                    t_idx = (c0 + sub * 128) // 128
                    ps = psmm.tile([128, F1], mybir.dt.float32, tag="mm1")
                    for kc in range(KC1):
                        nc.tensor.matmul(ps[:], xt[:, kc, sub * 128:(sub + 1) * 128],
                                         w1_t[:, kc, :], start=(kc == 0), stop=(kc == KC1 - 1))
                    ob = wp.tile([128, F1], mybir.dt.bfloat16, tag="m1o")
                    nc.vector.tensor_scalar(ob[:], ps[:], cs_t[:, t_idx:t_idx + 1], None, AL.mult)
                    m_store(1, t_idx, ob, F1)

            # ---- agg helper: gather + on-chip sel + sel-stationary matmul ----
            def agg_layer(li, elem, fwidth, finish_tile):
                cur = {}
                for call in range(N_CALLS):
                    g = gp.tile([128, CPC, elem], mybir.dt.bfloat16, tag=f"g{elem}")
                    nc.gpsimd.dma_gather(
                        g[:], m_full[li][IDX_OFF:, :],
                        idx_t[:, call * (CALL // 16):(call + 1) * (CALL // 16)],
                        CALL, CALL, elem, queue_num=call % 4)
                    selg = selp.tile([128, CPC, 128], mybir.dt.bfloat16, tag="selg")
                    ch0 = call * CPC
                    nc.vector.tensor_tensor(
                        selg[:], iota_t[:],
                        dstv_t[:, ch0:ch0 + CPC].unsqueeze(2).broadcast_to([128, CPC, 128]),
                        AL.is_equal)
                    for j in range(CPC):
                        ch = ch0 + j
                        t_idx = ch // R_CHUNKS
                        first = (ch % R_CHUNKS == 0)
                        last = (ch % R_CHUNKS == R_CHUNKS - 1)
                        if first:
                            cur[0] = psA.tile([128, fwidth], mybir.dt.float32, tag="aggA", name="psagg")
                        nc.tensor.matmul(cur[0][:], selg[:, j, :], g[:, j, 0:fwidth],
                                         start=first, stop=last)
                        if last:
                            finish_tile(cur[0], t_idx)
                            cur.clear()

            # ---- layer 1 agg -> H1T; fused L2 dense + chunked AG2 ----
            def l1_tile(ps_agg, t_idx):
                sl = slice(t_idx * 128, (t_idx + 1) * 128)
                h1d = wp.tile([128, F1], mybir.dt.bfloat16, tag="h1d")
                nc.vector.scalar_tensor_tensor(
                    h1d[:], ps_agg[:], cdpp_t[:, t_idx:t_idx + 1], b1r_t[:],
                    AL.mult, AL.add)
                nc.scalar.activation(h1d[:], h1d[:], AF.Relu)
                for fc in range(2):
                    trp = psT.tile([128, 128], mybir.dt.bfloat16, tag="tr")
                    nc.tensor.transpose(trp[:], h1d[:, fc * 128:(fc + 1) * 128], ident_t[:])
                    nc.scalar.activation(h1t[:, fc, sl], trp[:], AF.Copy)
                # fused L2 dense for this tile
                ps2 = psmm.tile([128, F1], mybir.dt.float32, tag="mm1", name="ps2")
                for fc in range(2):
                    nc.tensor.matmul(ps2[:, 0:F2], h1t[:, fc, sl], w2_t[:, fc, :],
                                     start=(fc == 0), stop=(fc == 1))
                ob2 = wp.tile([128, F2], mybir.dt.bfloat16, tag="ob2")
                nc.vector.tensor_scalar(ob2[:], ps2[:, 0:F2], cs_t[:, t_idx:t_idx + 1], None, AL.mult)
                m_store(2, t_idx, ob2, F2)

            agg_layer(1, F1, F1, l1_tile)

            # ---- layer 2 agg -> H2T; fused L3 dense + chunked AG3 ----
            def l2_tile(ps_agg, t_idx):
                sl = slice(t_idx * 128, (t_idx + 1) * 128)
                h2d = wp.tile([128, F2], mybir.dt.bfloat16, tag="h2d")
                nc.vector.scalar_tensor_tensor(
                    h2d[:], ps_agg[:], cdpp_t[:, t_idx:t_idx + 1], b2r_t[:],
                    AL.mult, AL.add)
                nc.scalar.activation(h2d[:], h2d[:], AF.Relu)
                trp = psT.tile([128, 128], mybir.dt.bfloat16, tag="tr", name="trp2")
                nc.tensor.transpose(trp[0:F2, :], h2d[:], ident_t[:])
                nc.scalar.activation(h2t[:, sl], trp[0:F2, :], AF.Copy)
                # fused L3 dense for this tile
                ps3 = psmm.tile([128, F1], mybir.dt.float32, tag="mm1", name="ps3")
                nc.tensor.matmul(ps3[:, 0:F3], h2t[:, sl], w3_t[:], start=True, stop=True)
                ob3 = wp.tile([128, F3], mybir.dt.bfloat16, tag="ob3")
                nc.vector.tensor_scalar(ob3[:], ps3[:, 0:F3], cs_t[:, t_idx:t_idx + 1], None, AL.mult)
                m_store(3, t_idx, ob3, F3)

            agg_layer(2, FPAD, F2, l2_tile)

            # ---- layer 3 agg -> per-tile log_softmax -> out ----
            def l3_tile(ps_agg, t_idx):
                xs = wp.tile([128, F3], mybir.dt.float32, tag="xs")
                nc.vector.scalar_tensor_tensor(
                    xs[:], ps_agg[:], cdpp_t[:, t_idx:t_idx + 1], b3r_t[:],
                    AL.mult, AL.add)
                ex = wp.tile([128, F3], mybir.dt.float32, tag="ex")
                nc.scalar.activation(ex[:], xs[:], AF.Exp)
                sm = wp.tile([128, 1], mybir.dt.float32, tag="sm")
                nc.vector.tensor_reduce(sm[:], ex[:], mybir.AxisListType.X, AL.add)
                rs = wp.tile([128, 1], mybir.dt.float32, tag="rs")
                nc.vector.reciprocal(rs[:], sm[:])
                ln = wp.tile([128, 1], mybir.dt.float32, tag="ln")
                nc.scalar.activation(ln[:], rs[:], AF.Ln)
                ox = wp.tile([128, F3], mybir.dt.float32, tag="ox")
                nc.vector.tensor_scalar(ox[:], xs[:], ln[:, 0:1], None, AL.add)
                nc.sync.dma_start(out_t[t_idx * 128:(t_idx + 1) * 128, :], ox[:])

            agg_layer(3, FPAD, F3, l3_tile)

    nc.compile()
    return nc


def _install_profile_shim():
    """Provide the missing antenv.axon_hooks module so trace=True works under axon."""
    try:
        import types
        import antenv
        if 'antenv.axon_hooks' in sys.modules:
            return
        _hook = [None]
        mod = types.ModuleType('antenv.axon_hooks')
        mod.set_axon_ntff_profile_hook = lambda h: _hook.__setitem__(0, h)
        mod.get_axon_ntff_profile_hook = lambda: _hook[0]
        sys.modules['antenv.axon_hooks'] = mod
        antenv.axon_hooks = mod
        from trn_agent_boot.trn_boot import _ntff_profile_via_ctypes
        mod.set_axon_ntff_profile_hook(
            _ntff_profile_via_ctypes('/opt/axon/libaxon_pjrt.so'))
    except Exception:
        pass


_CACHE = {}


def kernel(features, edge_index, W1, b1, W2, b2, W3, b3):
    global last_exec_time_ns
    features = np.asarray(features, dtype=np.float32)
    pre = _preprocess(np.asarray(edge_index))

    if 'nc' not in _CACHE:
        _CACHE['nc'] = _build_nc()
    nc = _CACHE['nc']

    # host-side input prep
    W1p = np.zeros((F_IN_P, F1), dtype=BF16)
    W1p[:F_IN] = np.asarray(W1, dtype=BF16)
    W2b = np.asarray(W2, dtype=BF16)
    W3b = np.asarray(W3, dtype=BF16)
    b1r = np.tile(np.asarray(b1, dtype=np.float32), (128, 1))
    b2r = np.tile(np.asarray(b2, dtype=np.float32), (128, 1))
    b3r = np.tile(np.asarray(b3, dtype=np.float32), (128, 1))
    iota = np.ascontiguousarray(np.broadcast_to(
        np.arange(128, dtype=np.float32), (128, CPC, 128))).astype(BF16)
    ident = np.eye(128, dtype=BF16)

    # features, permuted and transposed per core: [F_IN_P, 6272] bf16
    feat_b = features.astype(BF16)
    in_maps = []
    for c in range(N_CORES):
        rows = pre['node_of_row'][c * ROWS_PER_CORE:(c + 1) * ROWS_PER_CORE]
        xTc = np.zeros((F_IN_P, ROWS_PER_CORE), dtype=BF16)
        real = rows >= 0
        xTc[:F_IN, real] = feat_b[rows[real]].T
        in_maps.append({
            'xT': xTc, 'W1': W1p, 'W2': W2b, 'W3': W3b,
            'idx': pre['idx_tile'][c], 'dstv': pre['dstv'][c],
            'iota': iota, 'ident': ident,
            'cdst_pp': pre['cdst_pp'][c], 'csrc_t': pre['csrc_t'][c],
            'b1r': b1r, 'b2r': b2r, 'b3r': b3r,
        })

    trace = os.environ.get('BASS_KERNEL_TRACE', '0') == '1'
    if trace:
        _install_profile_shim()
    res = run_bass_kernel_spmd(nc, in_maps, core_ids=list(range(N_CORES)), trace=trace)
    last_exec_time_ns = res.exec_time_ns

    # assemble + inverse permute
    out_rows = np.concatenate([res.results[c]['out'] for c in range(N_CORES)], axis=0)
    out = np.empty((N_NODES, F3), dtype=np.float32)
    real = pre['node_of_row'] >= 0
    out[pre['node_of_row'][real]] = out_rows[real]
    return out


# revision 12
# speedup vs baseline: 1.3569x; 1.0302x over previous
"""Trainium2 Bass kernel for nn_DGL_Net (3-layer GraphConv GNN, 50000 nodes, 800k edges).

Strategy (8 NeuronCores, SPMD):
  - Host: relabel nodes into 392 balanced tiles of 128 nodes (<=2046 in-edges per
    tile), 49 tiles per core. Per layer: local matmul (bf16) -> scale by c_src ->
    AllGather (row-chunked, overlapped) -> per-edge dma_gather (4 SWDGE queues,
    src-sorted within each tile for HBM locality) -> one-hot (Sel) matmul
    aggregation in PSUM -> scale by c_dst + bias (+relu / log_softmax).
  - Sel one-hot matrices are generated ON-CHIP (one DVE is_equal per 1024-edge
    gather call, comparing a resident iota tile against broadcast dst lanes)
    instead of streaming 25.7MB/layer of precomputed one-hots from HBM.
  - Aggregation matmul is sel-stationary: PSUM[d,f] += Sel[e,d].T @ G[e,f] per
    128-edge chunk (one matmul per chunk). Per-tile epilogue applies
    c_dst (per-partition scalar) + bias + relu, PE-transposes back to [f,d]
    layout, and immediately runs the NEXT layer's dense matmul for that tile
    so the AllGathers can start early (chunked, overlapped with compute).
  - int16 gather indices: gather base is offset +32768 rows so idx = row-32768
    spans the whole [0, 50176) row space within int16. The last slot of every
    1024-index gather call is a reserved dummy with idx>=0 (defeats the ucode's
    trailing-negative trim).
"""
import os
import sys

sys.path.insert(0, '/opt/trn_rl_repo')

import numpy as np
import ml_dtypes

import concourse.bass as bass
import concourse.bacc as bacc
import concourse.mybir as mybir
import concourse.tile as tile
from concourse.bass_utils import run_bass_kernel_spmd

BF16 = ml_dtypes.bfloat16

N_NODES = 50000
N_CORES = 8
TILE_N = 128                 # nodes per tile
TILES_PER_CORE = 49
N_TILES = N_CORES * TILES_PER_CORE      # 392
ROWS_PER_CORE = TILES_PER_CORE * TILE_N  # 6272
N_ROWS = N_CORES * ROWS_PER_CORE         # 50176
R_CHUNKS = 16                # edge chunks (of 128 slots) per tile
SLOTS_PER_TILE = R_CHUNKS * 128          # 2048
TILE_EDGE_CAP = SLOTS_PER_TILE - 2       # 2046 (2 reserved call-end dummies)
SLOTS = TILES_PER_CORE * SLOTS_PER_TILE  # 100352 per core
CALL = 1024                  # idxs per dma_gather call
CPC = CALL // 128            # chunks per call (8)
N_CALLS = SLOTS // CALL      # 98
CHUNKS = TILES_PER_CORE * R_CHUNKS       # 784 chunks per core
IDX_OFF = 32768              # gather base offset (int16 trick)
F_IN = 1433
F_IN_P = 1536                # padded to 12*128
KC1 = F_IN_P // 128          # 12
F1 = 256
F2 = 32
F3 = 7
FPAD = 128                   # padded row width for M2/M3 gather (256B elems)
# AllGather row segments (tile counts); fired as each segment's tiles finish.
# The last segment is a single tile so the final (exposed) AllGather is tiny.
SEG_T = [16, 16, 16, 1]
SEG_R = [t * TILE_N for t in SEG_T]               # [2048, 2048, 2048, 128]
SEG_START = [0, 2048, 4096, 6144]                 # per-core local row starts
SEG_FULL = [0, 16384, 32768, 49152]               # chunk-major full-table starts
AG_FIRE = {15: 0, 31: 1, 47: 2, 48: 3}            # t_idx -> segment to fire

last_exec_time_ns = None


def _preprocess(edge_index):
    """Graph preprocessing: normalization constants, node->($core,tile,lane)
    relabeling with balanced per-tile in-degree, per-core edge slot tables
    (slots sorted by src row within each tile for gather locality)."""
    src = np.asarray(edge_index[0], dtype=np.int64)
    dst = np.asarray(edge_index[1], dtype=np.int64)
    n_edges = src.shape[0]

    deg_out = np.bincount(src, minlength=N_NODES).astype(np.float64)
    deg_in = np.bincount(dst, minlength=N_NODES).astype(np.float64)
    c_src = (1.0 / np.sqrt(np.maximum(deg_out, 1.0))).astype(np.float32)
    c_dst = (1.0 / np.sqrt(np.maximum(deg_in, 1.0))).astype(np.float32)

    # --- greedy balanced tile packing by in-degree ---
    import heapq
    order = np.argsort(-deg_in, kind='stable')
    heap = [(0.0, 0, t) for t in range(N_TILES)]  # (load, count, tile)
    heapq.heapify(heap)
    tile_nodes = [[] for _ in range(N_TILES)]
    tile_load = np.zeros(N_TILES)
    deferred = []
    for v in order:
        dv = deg_in[v]
        while True:
            load, cnt, t = heapq.heappop(heap)
            if cnt >= TILE_N:
                continue  # stale/full
            if load + dv > TILE_EDGE_CAP:
                deferred.append((load, cnt, t))
                continue
            break
        tile_nodes[t].append(int(v))
        tile_load[t] = load + dv
        heapq.heappush(heap, (load + dv, cnt + 1, t))
        for item in deferred:
            heapq.heappush(heap, item)
        deferred = []
    assert max(tile_load) <= TILE_EDGE_CAP

    # sort tiles by load desc, group by 8, core c takes c-th of each group
    tsort = np.argsort(-tile_load, kind='stable')
    tile_assign = np.empty((N_CORES, TILES_PER_CORE), dtype=np.int64)
    for k in range(TILES_PER_CORE):
        for c in range(N_CORES):
            tile_assign[c, k] = tsort[k * N_CORES + c]

    # row mapping: row = c*ROWS_PER_CORE + k*128 + lane
    row_of_node = np.full(N_NODES, -1, dtype=np.int64)
    node_of_row = np.full(N_ROWS, -1, dtype=np.int64)  # -1 = virtual pad node
    for c in range(N_CORES):
        for k in range(TILES_PER_CORE):
            t = tile_assign[c, k]
            nodes = tile_nodes[t]
            base = c * ROWS_PER_CORE + k * TILE_N
            for lane, v in enumerate(nodes):
                row_of_node[v] = base + lane
                node_of_row[base + lane] = v
    assert (row_of_node >= 0).all()

    # --- per-core edge slot tables ---
    dst_row = row_of_node[dst]
    src_row = row_of_node[src]
    e_core = dst_row // ROWS_PER_CORE
    e_tile = (dst_row % ROWS_PER_CORE) // TILE_N   # k within core
    e_lane = dst_row % TILE_N

    idx_flat = np.zeros((N_CORES, SLOTS), dtype=np.int16)      # pad idx = 0
    dst_flat = np.full((N_CORES, SLOTS), -1, dtype=np.int16)   # pad dst = -1

    # gather-row renumbering: full tables are laid out chunk-major
    # ([all cores' seg-0 rows, then seg-1, ...]) so each chunked AllGather
    # output is contiguous
    seg_start = np.asarray(SEG_START)
    seg_r = np.asarray(SEG_R)
    seg_full = np.asarray(SEG_FULL)
    sc = src_row // ROWS_PER_CORE
    sr = src_row % ROWS_PER_CORE
    seg_i = np.searchsorted(seg_start, sr, side='right') - 1
    src_grow = seg_full[seg_i] + sc * seg_r[seg_i] + (sr - seg_start[seg_i])

    # group edges by (core, tile); within each tile sort by src row (gather
    # locality), then assign slot positions skipping reserved slots 1023/2047
    key = e_core * TILES_PER_CORE + e_tile
    eorder = np.lexsort((src_grow, key))   # sort by key, then src gather-row
    key_s = key[eorder]
    grp_start = np.searchsorted(key_s, np.arange(N_CORES * TILES_PER_CORE))
    pos_in_grp = np.arange(n_edges) - grp_start[key_s]
    j = pos_in_grp
    slot_in_tile = j + (j >= 1023).astype(np.int64)  # j>=1023 shifts past slot 1023
    assert slot_in_tile.max() < SLOTS_PER_TILE - 1   # never hits 2047
    slots_abs = key_s % TILES_PER_CORE * SLOTS_PER_TILE + slot_in_tile
    cores_s = key_s // TILES_PER_CORE
    idx_flat[cores_s, slots_abs] = (src_grow[eorder] - IDX_OFF).astype(np.int16)
    dst_flat[cores_s, slots_abs] = e_lane[eorder].astype(np.int16)

    # wrap idx to [128, SLOTS/16] (idx i -> [i%16 replicated, i//16])
    cols = SLOTS // 16
    idx_tile = np.zeros((N_CORES, 128, cols), dtype=np.int16)
    for c in range(N_CORES):
        w = idx_flat[c].reshape(cols, 16).T  # [16, cols]
        idx_tile[c] = np.tile(w, (8, 1))

    # dst lane per slot, wrapped [128 lanes, CHUNKS] bf16 (for on-chip sel-gen)
    dstv = np.empty((N_CORES, 128, CHUNKS), dtype=BF16)
    for c in range(N_CORES):
        dstv[c] = dst_flat[c].reshape(CHUNKS, 128).T.astype(BF16)

    # per-core normalization tables
    cd_row = np.where(node_of_row >= 0, c_dst[np.maximum(node_of_row, 0)], 1.0)
    cs_row = np.where(node_of_row >= 0, c_src[np.maximum(node_of_row, 0)], 1.0)
    cd_core = cd_row.reshape(N_CORES, ROWS_PER_CORE).astype(np.float32)
    cs_core = cs_row.reshape(N_CORES, ROWS_PER_CORE).astype(np.float32)
    cdst_pp = cd_core.reshape(N_CORES, TILES_PER_CORE, 128).transpose(0, 2, 1).copy()
    csrc_t = cs_core.reshape(N_CORES, TILES_PER_CORE, 128).transpose(0, 2, 1).copy()

    return dict(row_of_node=row_of_node, node_of_row=node_of_row,
                idx_tile=idx_tile, dstv=dstv,
                cdst_pp=cdst_pp, csrc_t=csrc_t)


def _build_nc():
    nc = bacc.Bacc("TRN2", target_bir_lowering=False, debug=False,
                   enable_asserts=True, num_devices=N_CORES, num_swdge_queues=4)
    dt = mybir.dt
    inp = {}
    inp['xT'] = nc.dram_tensor("xT", [F_IN_P, ROWS_PER_CORE], dt.bfloat16, kind="ExternalInput")
    inp['W1'] = nc.dram_tensor("W1", [F_IN_P, F1], dt.bfloat16, kind="ExternalInput")
    inp['W2'] = nc.dram_tensor("W2", [F1, F2], dt.bfloat16, kind="ExternalInput")
    inp['W3'] = nc.dram_tensor("W3", [F2, F3], dt.bfloat16, kind="ExternalInput")
    inp['idx'] = nc.dram_tensor("idx", [128, SLOTS // 16], dt.int16, kind="ExternalInput")
    inp['dstv'] = nc.dram_tensor("dstv", [128, CHUNKS], dt.bfloat16, kind="ExternalInput")
    inp['iota'] = nc.dram_tensor("iota", [128, CPC, 128], dt.bfloat16, kind="ExternalInput")
    inp['ident'] = nc.dram_tensor("ident", [128, 128], dt.bfloat16, kind="ExternalInput")
    inp['cdst_pp'] = nc.dram_tensor("cdst_pp", [128, TILES_PER_CORE], dt.float32, kind="ExternalInput")
    inp['csrc_t'] = nc.dram_tensor("csrc_t", [128, TILES_PER_CORE], dt.float32, kind="ExternalInput")
    inp['b1r'] = nc.dram_tensor("b1r", [128, F1], dt.float32, kind="ExternalInput")
    inp['b2r'] = nc.dram_tensor("b2r", [128, F2], dt.float32, kind="ExternalInput")
    inp['b3r'] = nc.dram_tensor("b3r", [128, F3], dt.float32, kind="ExternalInput")
    out_t = nc.dram_tensor("out", [ROWS_PER_CORE, F3], dt.float32, kind="ExternalOutput")

    m_own = {}
    for li, w in ((1, F1), (2, FPAD), (3, FPAD)):
        for sg in range(4):
            m_own[li, sg] = nc.dram_tensor(f"m{li}_own_{sg}", [SEG_R[sg], w], dt.bfloat16)
    m_full = {
        1: nc.dram_tensor("m1_full", [N_ROWS, F1], dt.bfloat16, addr_space="Shared"),
        2: nc.dram_tensor("m2_full", [N_ROWS, FPAD], dt.bfloat16, addr_space="Shared"),
        3: nc.dram_tensor("m3_full", [N_ROWS, FPAD], dt.bfloat16, addr_space="Shared"),
    }

    AL = mybir.AluOpType
    AF = mybir.ActivationFunctionType
    RG = [list(range(N_CORES))]

    def ag_seg(li, sg):
        """AllGather segment sg of layer li's table (contiguous chunk-major rows)."""
        lo = SEG_FULL[sg]
        hi = lo + N_CORES * SEG_R[sg]
        nc.gpsimd.collective_compute(
            "AllGather", AL.bypass, replica_groups=RG,
            ins=[m_own[li, sg][:, :]], outs=[m_full[li][lo:hi, :]])

    def m_store(li, t_idx, ob, w):
        sg = int(np.searchsorted(np.asarray(SEG_START), t_idx * 128, side='right')) - 1
        r = t_idx * 128 - SEG_START[sg]
        nc.sync.dma_start(m_own[li, sg][r:r + 128, 0:w], ob[:])
        if t_idx in AG_FIRE:
            ag_seg(li, AG_FIRE[t_idx])

    with tile.TileContext(nc) as tc:
        with tc.tile_pool(name="const", bufs=1) as constp, \
             tc.tile_pool(name="big", bufs=1) as bigp, \
             tc.tile_pool(name="xstream", bufs=2) as xp, \
             tc.tile_pool(name="work", bufs=4) as wp, \
             tc.tile_pool(name="gpool", bufs=4) as gp, \
             tc.tile_pool(name="selp", bufs=4) as selp, \
             tc.tile_pool(name="psA", bufs=2, space="PSUM") as psA, \
             tc.tile_pool(name="psT", bufs=2, space="PSUM") as psT, \
             tc.tile_pool(name="psmm", bufs=2, space="PSUM") as psmm:

            # ---- resident constants ----
            w1_t = constp.tile([128, KC1, F1], mybir.dt.bfloat16)
            nc.sync.dma_start(w1_t[:], inp['W1'].rearrange("(kc p) n -> p kc n", p=128))
            w2_t = constp.tile([128, 2, F2], mybir.dt.bfloat16)
            nc.sync.dma_start(w2_t[:], inp['W2'].rearrange("(kc p) n -> p kc n", p=128))
            w3_t = constp.tile([F2, F3], mybir.dt.bfloat16)
            nc.sync.dma_start(w3_t[:], inp['W3'][:, :])
            idx_t = constp.tile([128, SLOTS // 16], mybir.dt.int16)
            nc.sync.dma_start(idx_t[:], inp['idx'][:, :])
            dstv_t = constp.tile([128, CHUNKS], mybir.dt.bfloat16)
            nc.sync.dma_start(dstv_t[:], inp['dstv'][:, :])
            iota_t = constp.tile([128, CPC, 128], mybir.dt.bfloat16)
            nc.sync.dma_start(iota_t[:], inp['iota'][:, :, :])
            ident_t = constp.tile([128, 128], mybir.dt.bfloat16)
            nc.sync.dma_start(ident_t[:], inp['ident'][:, :])
            cdpp_t = constp.tile([128, TILES_PER_CORE], mybir.dt.float32)
            nc.sync.dma_start(cdpp_t[:], inp['cdst_pp'][:, :])
            cs_t = constp.tile([128, TILES_PER_CORE], mybir.dt.float32)
            nc.sync.dma_start(cs_t[:], inp['csrc_t'][:, :])
            b1r_t = constp.tile([128, F1], mybir.dt.float32)
            nc.sync.dma_start(b1r_t[:], inp['b1r'][:, :])
            b2r_t = constp.tile([128, F2], mybir.dt.float32)
            nc.sync.dma_start(b2r_t[:], inp['b2r'][:, :])
            b3r_t = constp.tile([128, F3], mybir.dt.float32)
            nc.sync.dma_start(b3r_t[:], inp['b3r'][:, :])

            h1t = bigp.tile([128, 2, ROWS_PER_CORE], mybir.dt.bfloat16)  # H1.T
            h2t = bigp.tile([F2, ROWS_PER_CORE], mybir.dt.bfloat16)      # H2.T

            # ---- phase 1: M1 = (X @ W1) * c_src, row-chunked AllGather ----
            blocks = [(i * 512, 512) for i in range(12)] + [(6144, 128)]
            for c0, bs in blocks:
                xt = xp.tile([128, KC1, bs], mybir.dt.bfloat16, tag="xt")
                nc.sync.dma_start(
                    xt[:, :, :bs],
                    inp['xT'][:, c0:c0 + bs].rearrange("(kc p) n -> p kc n", p=128))
                for sub in range(bs // 128):
                    t_idx = (c0 + sub * 128) // 128
                    ps = psmm.tile([128, F1], mybir.dt.float32, tag="mm1")
                    for kc in range(KC1):
                        nc.tensor.matmul(ps[:], xt[:, kc, sub * 128:(sub + 1) * 128],
                                         w1_t[:, kc, :], start=(kc == 0), stop=(kc == KC1 - 1))
                    ob = wp.tile([128, F1], mybir.dt.bfloat16, tag="m1o")
                    nc.vector.tensor_scalar(ob[:], ps[:], cs_t[:, t_idx:t_idx + 1], None, AL.mult)
                    m_store(1, t_idx, ob, F1)

            # ---- agg helper: gather + on-chip sel + sel-stationary matmul ----
            def agg_layer(li, elem, fwidth, finish_tile):
                cur = {}
                for call in range(N_CALLS):
                    g = gp.tile([128, CPC, elem], mybir.dt.bfloat16, tag=f"g{elem}")
                    nc.gpsimd.dma_gather(
                        g[:], m_full[li][IDX_OFF:, :],
                        idx_t[:, call * (CALL // 16):(call + 1) * (CALL // 16)],
                        CALL, CALL, elem, queue_num=call % 4)
                    selg = selp.tile([128, CPC, 128], mybir.dt.bfloat16, tag="selg")
                    ch0 = call * CPC
                    nc.vector.tensor_tensor(
                        selg[:], iota_t[:],
                        dstv_t[:, ch0:ch0 + CPC].unsqueeze(2).broadcast_to([128, CPC, 128]),
                        AL.is_equal)
                    for j in range(CPC):
                        ch = ch0 + j
                        t_idx = ch // R_CHUNKS
                        first = (ch % R_CHUNKS == 0)
                        last = (ch % R_CHUNKS == R_CHUNKS - 1)
                        if first:
                            cur[0] = psA.tile([128, fwidth], mybir.dt.float32, tag="aggA", name="psagg")
                        nc.tensor.matmul(cur[0][:], selg[:, j, :], g[:, j, 0:fwidth],
                                         start=first, stop=last)
                        if last:
                            finish_tile(cur[0], t_idx)
                            cur.clear()

            # ---- layer 1 agg -> H1T; fused L2 dense + chunked AG2 ----
            def l1_tile(ps_agg, t_idx):
                sl = slice(t_idx * 128, (t_idx + 1) * 128)
                h1d = wp.tile([128, F1], mybir.dt.bfloat16, tag="h1d")
                nc.vector.scalar_tensor_tensor(
                    h1d[:], ps_agg[:], cdpp_t[:, t_idx:t_idx + 1], b1r_t[:],
                    AL.mult, AL.add)
                nc.scalar.activation(h1d[:], h1d[:], AF.Relu)
                for fc in range(2):
                    trp = psT.tile([128, 128], mybir.dt.bfloat16, tag="tr")
                    nc.tensor.transpose(trp[:], h1d[:, fc * 128:(fc + 1) * 128], ident_t[:])
                    nc.scalar.activation(h1t[:, fc, sl], trp[:], AF.Copy)
                # fused L2 dense for this tile
                ps2 = psmm.tile([128, F1], mybir.dt.float32, tag="mm1", name="ps2")
                for fc in range(2):
                    nc.tensor.matmul(ps2[:, 0:F2], h1t[:, fc, sl], w2_t[:, fc, :],
                                     start=(fc == 0), stop=(fc == 1))
                ob2 = wp.tile([128, F2], mybir.dt.bfloat16, tag="ob2")
                nc.vector.tensor_scalar(ob2[:], ps2[:, 0:F2], cs_t[:, t_idx:t_idx + 1], None, AL.mult)
                m_store(2, t_idx, ob2, F2)

            agg_layer(1, F1, F1, l1_tile)

            # ---- layer 2 agg -> H2T; fused L3 dense + chunked AG3 ----
            def l2_tile(ps_agg, t_idx):
                sl = slice(t_idx * 128, (t_idx + 1) * 128)
                h2d = wp.tile([128, F2], mybir.dt.bfloat16, tag="h2d")
                nc.vector.scalar_tensor_tensor(
                    h2d[:], ps_agg[:], cdpp_t[:, t_idx:t_idx + 1], b2r_t[:],
                    AL.mult, AL.add)
                nc.scalar.activation(h2d[:], h2d[:], AF.Relu)
                trp = psT.tile([128, 128], mybir.dt.bfloat16, tag="tr", name="trp2")
                nc.tensor.transpose(trp[0:F2, :], h2d[:], ident_t[:])
                nc.scalar.activation(h2t[:, sl], trp[0:F2, :], AF.Copy)
                # fused L3 dense for this tile
                ps3 = psmm.tile([128, F1], mybir.dt.float32, tag="mm1", name="ps3")
                nc.tensor.matmul(ps3[:, 0:F3], h2t[:, sl], w3_t[:], start=True, stop=True)
                ob3 = wp.tile([128, F3], mybir.dt.bfloat16, tag="ob3")
                nc.vector.tensor_scalar(ob3[:], ps3[:, 0:F3], cs_t[:, t_idx:t_idx + 1], None, AL.mult)
                m_store(3, t_idx, ob3, F3)

            agg_layer(2, FPAD, F2, l2_tile)

            # ---- layer 3 agg -> logits; softmax batched in two halves ----
            xall = bigp.tile([128, TILES_PER_CORE * F3], mybir.dt.float32)

            def softmax_part(tlo, thi):
                flo, fhi = tlo * F3, thi * F3
                exa = wp.tile([128, (thi - tlo) * F3], mybir.dt.float32,
                              tag="exa", name="exa")
                nc.scalar.activation(exa[:], xall[:, flo:fhi], AF.Exp)
                smv = wp.tile([128, thi - tlo], mybir.dt.float32, tag="smv", name="smv")
                nc.vector.tensor_reduce(
                    smv[:], exa[:].rearrange("p (t f) -> p t f", f=F3),
                    mybir.AxisListType.X, AL.add)
                rsv = wp.tile([128, thi - tlo], mybir.dt.float32, tag="rsv", name="rsv")
                nc.vector.reciprocal(rsv[:], smv[:])
                nlog = wp.tile([128, thi - tlo], mybir.dt.float32, tag="nlog", name="nlog")
                nc.scalar.activation(nlog[:], rsv[:], AF.Ln)
                for t_idx in range(tlo, thi):
                    sl3 = slice(t_idx * F3, (t_idx + 1) * F3)
                    ox = wp.tile([128, F3], mybir.dt.float32, tag="ox")
                    nc.vector.tensor_scalar(ox[:], xall[:, sl3],
                                            nlog[:, t_idx - tlo:t_idx - tlo + 1], None, AL.add)
                    nc.sync.dma_start(out_t[t_idx * 128:(t_idx + 1) * 128, :], ox[:])

            def l3_tile(ps_agg, t_idx):
                sl3 = slice(t_idx * F3, (t_idx + 1) * F3)
                nc.vector.scalar_tensor_tensor(
                    xall[:, sl3], ps_agg[:], cdpp_t[:, t_idx:t_idx + 1], b3r_t[:],
                    AL.mult, AL.add)
                if t_idx == 23:
                    softmax_part(0, 24)

            agg_layer(3, FPAD, F3, l3_tile)
            softmax_part(24, TILES_PER_CORE)

    nc.compile()
    return nc


def _install_profile_shim():
    """Provide the missing antenv.axon_hooks module so trace=True works under axon."""
    try:
        import types
        import antenv
        if 'antenv.axon_hooks' in sys.modules:
            return
        _hook = [None]
        mod = types.ModuleType('antenv.axon_hooks')
        mod.set_axon_ntff_profile_hook = lambda h: _hook.__setitem__(0, h)
        mod.get_axon_ntff_profile_hook = lambda: _hook[0]
        sys.modules['antenv.axon_hooks'] = mod
        antenv.axon_hooks = mod
        from trn_agent_boot.trn_boot import _ntff_profile_via_ctypes
        mod.set_axon_ntff_profile_hook(
            _ntff_profile_via_ctypes('/opt/axon/libaxon_pjrt.so'))
    except Exception:
        pass


_CACHE = {}


def kernel(features, edge_index, W1, b1, W2, b2, W3, b3):
    global last_exec_time_ns
    features = np.asarray(features, dtype=np.float32)
    pre = _preprocess(np.asarray(edge_index))

    if 'nc' not in _CACHE:
        _CACHE['nc'] = _build_nc()
    nc = _CACHE['nc']

    # host-side input prep
    W1p = np.zeros((F_IN_P, F1), dtype=BF16)
    W1p[:F_IN] = np.asarray(W1, dtype=BF16)
    W2b = np.asarray(W2, dtype=BF16)
    W3b = np.asarray(W3, dtype=BF16)
    b1r = np.tile(np.asarray(b1, dtype=np.float32), (128, 1))
    b2r = np.tile(np.asarray(b2, dtype=np.float32), (128, 1))
    b3r = np.tile(np.asarray(b3, dtype=np.float32), (128, 1))
    iota = np.ascontiguousarray(np.broadcast_to(
        np.arange(128, dtype=np.float32), (128, CPC, 128))).astype(BF16)
    ident = np.eye(128, dtype=BF16)

    # features, permuted and transposed per core: [F_IN_P, 6272] bf16
    feat_b = features.astype(BF16)
    in_maps = []
    for c in range(N_CORES):
        rows = pre['node_of_row'][c * ROWS_PER_CORE:(c + 1) * ROWS_PER_CORE]
        xTc = np.zeros((F_IN_P, ROWS_PER_CORE), dtype=BF16)
        real = rows >= 0
        xTc[:F_IN, real] = feat_b[rows[real]].T
        in_maps.append({
            'xT': xTc, 'W1': W1p, 'W2': W2b, 'W3': W3b,
            'idx': pre['idx_tile'][c], 'dstv': pre['dstv'][c],
            'iota': iota, 'ident': ident,
            'cdst_pp': pre['cdst_pp'][c], 'csrc_t': pre['csrc_t'][c],
            'b1r': b1r, 'b2r': b2r, 'b3r': b3r,
        })

    trace = os.environ.get('BASS_KERNEL_TRACE', '0') == '1'
    if trace:
        _install_profile_shim()
    res = run_bass_kernel_spmd(nc, in_maps, core_ids=list(range(N_CORES)), trace=trace)
    last_exec_time_ns = res.exec_time_ns

    # assemble + inverse permute
    out_rows = np.concatenate([res.results[c]['out'] for c in range(N_CORES)], axis=0)
    out = np.empty((N_NODES, F3), dtype=np.float32)
    real = pre['node_of_row'] >= 0
    out[pre['node_of_row'][real]] = out_rows[real]
    return out


# revision 14
# speedup vs baseline: 1.3641x; 1.0053x over previous
"""Trainium2 Bass kernel for nn_DGL_Net (3-layer GraphConv GNN, 50000 nodes, 800k edges).

Strategy (8 NeuronCores, SPMD):
  - Host: relabel nodes into 392 balanced tiles of 128 nodes (<=2046 in-edges per
    tile), 49 tiles per core. Per layer: local matmul (bf16) -> scale by c_src ->
    AllGather (row-chunked, overlapped) -> per-edge dma_gather (4 SWDGE queues,
    src-sorted within each tile for HBM locality) -> one-hot (Sel) matmul
    aggregation in PSUM -> scale by c_dst + bias (+relu / log_softmax).
  - Sel one-hot matrices are generated ON-CHIP (one DVE is_equal per 1024-edge
    gather call, comparing a resident iota tile against broadcast dst lanes)
    instead of streaming 25.7MB/layer of precomputed one-hots from HBM.
  - Aggregation matmul is sel-stationary: PSUM[d,f] += Sel[e,d].T @ G[e,f] per
    128-edge chunk (one matmul per chunk). Per-tile epilogue applies
    c_dst (per-partition scalar) + bias + relu, PE-transposes back to [f,d]
    layout, and immediately runs the NEXT layer's dense matmul for that tile
    so the AllGathers can start early (chunked, overlapped with compute).
  - int16 gather indices: gather base is offset +32768 rows so idx = row-32768
    spans the whole [0, 50176) row space within int16. The last slot of every
    1024-index gather call is a reserved dummy with idx>=0 (defeats the ucode's
    trailing-negative trim).
"""
import os
import sys

sys.path.insert(0, '/opt/trn_rl_repo')

import numpy as np
import ml_dtypes

import concourse.bass as bass
import concourse.bacc as bacc
import concourse.mybir as mybir
import concourse.tile as tile
from concourse.bass_utils import run_bass_kernel_spmd

BF16 = ml_dtypes.bfloat16

N_NODES = 50000
N_CORES = 8
TILE_N = 128                 # nodes per tile
TILES_PER_CORE = 49
N_TILES = N_CORES * TILES_PER_CORE      # 392
ROWS_PER_CORE = TILES_PER_CORE * TILE_N  # 6272
N_ROWS = N_CORES * ROWS_PER_CORE         # 50176
R_CHUNKS = 16                # edge chunks (of 128 slots) per tile
SLOTS_PER_TILE = R_CHUNKS * 128          # 2048
TILE_EDGE_CAP = SLOTS_PER_TILE - 2       # 2046 (2 reserved call-end dummies)
SLOTS = TILES_PER_CORE * SLOTS_PER_TILE  # 100352 per core
CALL = 1024                  # idxs per dma_gather call
CPC = CALL // 128            # chunks per call (8)
N_CALLS = SLOTS // CALL      # 98
CHUNKS = TILES_PER_CORE * R_CHUNKS       # 784 chunks per core
IDX_OFF = 32768              # gather base offset (int16 trick)
F_IN = 1433
F_IN_P = 1536                # padded to 12*128
KC1 = F_IN_P // 128          # 12
F1 = 256
F2 = 32
F3 = 7
FPAD = 128                   # padded row width for M2/M3 gather (256B elems)
# AllGather row segments (tile counts); fired as each segment's tiles finish.
# The last segment is a single tile so the final (exposed) AllGather is tiny.
SEG_T = [16, 16, 15, 2]
SEG_R = [t * TILE_N for t in SEG_T]               # [2048, 2048, 1920, 256]
SEG_START = [0, 2048, 4096, 6016]                 # per-core local row starts
SEG_FULL = [0, 16384, 32768, 48128]               # chunk-major full-table starts
AG_FIRE = {15: 0, 31: 1, 46: 2, 48: 3}            # t_idx -> segment to fire

last_exec_time_ns = None


def _preprocess(edge_index):
    """Graph preprocessing: normalization constants, node->($core,tile,lane)
    relabeling with balanced per-tile in-degree, per-core edge slot tables
    (slots sorted by src row within each tile for gather locality)."""
    src = np.asarray(edge_index[0], dtype=np.int64)
    dst = np.asarray(edge_index[1], dtype=np.int64)
    n_edges = src.shape[0]

    deg_out = np.bincount(src, minlength=N_NODES).astype(np.float64)
    deg_in = np.bincount(dst, minlength=N_NODES).astype(np.float64)
    c_src = (1.0 / np.sqrt(np.maximum(deg_out, 1.0))).astype(np.float32)
    c_dst = (1.0 / np.sqrt(np.maximum(deg_in, 1.0))).astype(np.float32)

    # --- greedy balanced tile packing by in-degree ---
    import heapq
    order = np.argsort(-deg_in, kind='stable')
    heap = [(0.0, 0, t) for t in range(N_TILES)]  # (load, count, tile)
    heapq.heapify(heap)
    tile_nodes = [[] for _ in range(N_TILES)]
    tile_load = np.zeros(N_TILES)
    deferred = []
    for v in order:
        dv = deg_in[v]
        while True:
            load, cnt, t = heapq.heappop(heap)
            if cnt >= TILE_N:
                continue  # stale/full
            if load + dv > TILE_EDGE_CAP:
                deferred.append((load, cnt, t))
                continue
            break
        tile_nodes[t].append(int(v))
        tile_load[t] = load + dv
        heapq.heappush(heap, (load + dv, cnt + 1, t))
        for item in deferred:
            heapq.heappush(heap, item)
        deferred = []
    assert max(tile_load) <= TILE_EDGE_CAP

    # sort tiles by load desc, group by 8, core c takes c-th of each group
    tsort = np.argsort(-tile_load, kind='stable')
    tile_assign = np.empty((N_CORES, TILES_PER_CORE), dtype=np.int64)
    for k in range(TILES_PER_CORE):
        for c in range(N_CORES):
            tile_assign[c, k] = tsort[k * N_CORES + c]

    # row mapping: row = c*ROWS_PER_CORE + k*128 + lane
    row_of_node = np.full(N_NODES, -1, dtype=np.int64)
    node_of_row = np.full(N_ROWS, -1, dtype=np.int64)  # -1 = virtual pad node
    for c in range(N_CORES):
        for k in range(TILES_PER_CORE):
            t = tile_assign[c, k]
            nodes = tile_nodes[t]
            base = c * ROWS_PER_CORE + k * TILE_N
            for lane, v in enumerate(nodes):
                row_of_node[v] = base + lane
                node_of_row[base + lane] = v
    assert (row_of_node >= 0).all()

    # --- per-core edge slot tables ---
    dst_row = row_of_node[dst]
    src_row = row_of_node[src]
    e_core = dst_row // ROWS_PER_CORE
    e_tile = (dst_row % ROWS_PER_CORE) // TILE_N   # k within core
    e_lane = dst_row % TILE_N

    idx_flat = np.zeros((N_CORES, SLOTS), dtype=np.int16)      # pad idx = 0
    dst_flat = np.full((N_CORES, SLOTS), -1, dtype=np.int16)   # pad dst = -1

    # gather-row renumbering: full tables are laid out chunk-major
    # ([all cores' seg-0 rows, then seg-1, ...]) so each chunked AllGather
    # output is contiguous
    seg_start = np.asarray(SEG_START)
    seg_r = np.asarray(SEG_R)
    seg_full = np.asarray(SEG_FULL)
    sc = src_row // ROWS_PER_CORE
    sr = src_row % ROWS_PER_CORE
    seg_i = np.searchsorted(seg_start, sr, side='right') - 1
    src_grow = seg_full[seg_i] + sc * seg_r[seg_i] + (sr - seg_start[seg_i])

    # group edges by (core, tile); within each tile sort by src row (gather
    # locality), then assign slot positions skipping reserved slots 1023/2047
    key = e_core * TILES_PER_CORE + e_tile
    eorder = np.lexsort((src_grow, key))   # sort by key, then src gather-row
    key_s = key[eorder]
    grp_start = np.searchsorted(key_s, np.arange(N_CORES * TILES_PER_CORE))
    pos_in_grp = np.arange(n_edges) - grp_start[key_s]
    j = pos_in_grp
    slot_in_tile = j + (j >= 1023).astype(np.int64)  # j>=1023 shifts past slot 1023
    assert slot_in_tile.max() < SLOTS_PER_TILE - 1   # never hits 2047
    slots_abs = key_s % TILES_PER_CORE * SLOTS_PER_TILE + slot_in_tile
    cores_s = key_s // TILES_PER_CORE
    idx_flat[cores_s, slots_abs] = (src_grow[eorder] - IDX_OFF).astype(np.int16)
    dst_flat[cores_s, slots_abs] = e_lane[eorder].astype(np.int16)

    # wrap idx to [128, SLOTS/16] (idx i -> [i%16 replicated, i//16])
    cols = SLOTS // 16
    idx_tile = np.zeros((N_CORES, 128, cols), dtype=np.int16)
    for c in range(N_CORES):
        w = idx_flat[c].reshape(cols, 16).T  # [16, cols]
        idx_tile[c] = np.tile(w, (8, 1))

    # dst lane per slot, wrapped [128 lanes, CHUNKS] bf16 (for on-chip sel-gen)
    dstv = np.empty((N_CORES, 128, CHUNKS), dtype=BF16)
    for c in range(N_CORES):
        dstv[c] = dst_flat[c].reshape(CHUNKS, 128).T.astype(BF16)

    # per-core normalization tables
    cd_row = np.where(node_of_row >= 0, c_dst[np.maximum(node_of_row, 0)], 1.0)
    cs_row = np.where(node_of_row >= 0, c_src[np.maximum(node_of_row, 0)], 1.0)
    cd_core = cd_row.reshape(N_CORES, ROWS_PER_CORE).astype(np.float32)
    cs_core = cs_row.reshape(N_CORES, ROWS_PER_CORE).astype(np.float32)
    cdst_pp = cd_core.reshape(N_CORES, TILES_PER_CORE, 128).transpose(0, 2, 1).copy()
    csrc_t = cs_core.reshape(N_CORES, TILES_PER_CORE, 128).transpose(0, 2, 1).copy()

    return dict(row_of_node=row_of_node, node_of_row=node_of_row,
                idx_tile=idx_tile, dstv=dstv,
                cdst_pp=cdst_pp, csrc_t=csrc_t)


def _build_nc():
    nc = bacc.Bacc("TRN2", target_bir_lowering=False, debug=False,
                   enable_asserts=True, num_devices=N_CORES, num_swdge_queues=4)
    dt = mybir.dt
    inp = {}
    inp['xT'] = nc.dram_tensor("xT", [F_IN_P, ROWS_PER_CORE], dt.bfloat16, kind="ExternalInput")
    inp['W1'] = nc.dram_tensor("W1", [F_IN_P, F1], dt.bfloat16, kind="ExternalInput")
    inp['W2'] = nc.dram_tensor("W2", [F1, F2], dt.bfloat16, kind="ExternalInput")
    inp['W3'] = nc.dram_tensor("W3", [F2, F3], dt.bfloat16, kind="ExternalInput")
    inp['idx'] = nc.dram_tensor("idx", [128, SLOTS // 16], dt.int16, kind="ExternalInput")
    inp['dstv'] = nc.dram_tensor("dstv", [128, CHUNKS], dt.bfloat16, kind="ExternalInput")
    inp['iota'] = nc.dram_tensor("iota", [128, CPC, 128], dt.bfloat16, kind="ExternalInput")
    inp['ident'] = nc.dram_tensor("ident", [128, 128], dt.bfloat16, kind="ExternalInput")
    inp['cdst_pp'] = nc.dram_tensor("cdst_pp", [128, TILES_PER_CORE], dt.float32, kind="ExternalInput")
    inp['csrc_t'] = nc.dram_tensor("csrc_t", [128, TILES_PER_CORE], dt.float32, kind="ExternalInput")
    inp['b1r'] = nc.dram_tensor("b1r", [128, F1], dt.float32, kind="ExternalInput")
    inp['b2r'] = nc.dram_tensor("b2r", [128, F2], dt.float32, kind="ExternalInput")
    inp['b3r'] = nc.dram_tensor("b3r", [128, F3], dt.float32, kind="ExternalInput")
    out_t = nc.dram_tensor("out", [ROWS_PER_CORE, F3], dt.float32, kind="ExternalOutput")

    m_own = {}
    for li, w in ((1, F1), (2, FPAD), (3, FPAD)):
        for sg in range(4):
            m_own[li, sg] = nc.dram_tensor(f"m{li}_own_{sg}", [SEG_R[sg], w], dt.bfloat16)
    m_full = {
        1: nc.dram_tensor("m1_full", [N_ROWS, F1], dt.bfloat16, addr_space="Shared"),
        2: nc.dram_tensor("m2_full", [N_ROWS, FPAD], dt.bfloat16, addr_space="Shared"),
        3: nc.dram_tensor("m3_full", [N_ROWS, FPAD], dt.bfloat16, addr_space="Shared"),
    }

    AL = mybir.AluOpType
    AF = mybir.ActivationFunctionType
    RG = [list(range(N_CORES))]

    def ag_seg(li, sg):
        """AllGather segment sg of layer li's table (contiguous chunk-major rows)."""
        lo = SEG_FULL[sg]
        hi = lo + N_CORES * SEG_R[sg]
        nc.gpsimd.collective_compute(
            "AllGather", AL.bypass, replica_groups=RG,
            ins=[m_own[li, sg][:, :]], outs=[m_full[li][lo:hi, :]])

    def m_store(li, t_idx, ob, w):
        sg = int(np.searchsorted(np.asarray(SEG_START), t_idx * 128, side='right')) - 1
        r = t_idx * 128 - SEG_START[sg]
        nc.sync.dma_start(m_own[li, sg][r:r + 128, 0:w], ob[:])
        if t_idx in AG_FIRE:
            ag_seg(li, AG_FIRE[t_idx])

    with tile.TileContext(nc) as tc:
        with tc.tile_pool(name="const", bufs=1) as constp, \
             tc.tile_pool(name="big", bufs=1) as bigp, \
             tc.tile_pool(name="xstream", bufs=2) as xp, \
             tc.tile_pool(name="work", bufs=4) as wp, \
             tc.tile_pool(name="gpool", bufs=4) as gp, \
             tc.tile_pool(name="selp", bufs=4) as selp, \
             tc.tile_pool(name="psA", bufs=2, space="PSUM") as psA, \
             tc.tile_pool(name="psT", bufs=2, space="PSUM") as psT, \
             tc.tile_pool(name="psmm", bufs=2, space="PSUM") as psmm:

            # ---- resident constants ----
            w1_t = constp.tile([128, KC1, F1], mybir.dt.bfloat16)
            nc.sync.dma_start(w1_t[:], inp['W1'].rearrange("(kc p) n -> p kc n", p=128))
            w2_t = constp.tile([128, 2, F2], mybir.dt.bfloat16)
            nc.sync.dma_start(w2_t[:], inp['W2'].rearrange("(kc p) n -> p kc n", p=128))
            w3_t = constp.tile([F2, F3], mybir.dt.bfloat16)
            nc.sync.dma_start(w3_t[:], inp['W3'][:, :])
            idx_t = constp.tile([128, SLOTS // 16], mybir.dt.int16)
            nc.sync.dma_start(idx_t[:], inp['idx'][:, :])
            dstv_t = constp.tile([128, CHUNKS], mybir.dt.bfloat16)
            nc.sync.dma_start(dstv_t[:], inp['dstv'][:, :])
            iota_t = constp.tile([128, CPC, 128], mybir.dt.bfloat16)
            nc.sync.dma_start(iota_t[:], inp['iota'][:, :, :])
            ident_t = constp.tile([128, 128], mybir.dt.bfloat16)
            nc.sync.dma_start(ident_t[:], inp['ident'][:, :])
            cdpp_t = constp.tile([128, TILES_PER_CORE], mybir.dt.float32)
            nc.sync.dma_start(cdpp_t[:], inp['cdst_pp'][:, :])
            cs_t = constp.tile([128, TILES_PER_CORE], mybir.dt.float32)
            nc.sync.dma_start(cs_t[:], inp['csrc_t'][:, :])
            b1r_t = constp.tile([128, F1], mybir.dt.float32)
            nc.sync.dma_start(b1r_t[:], inp['b1r'][:, :])
            b2r_t = constp.tile([128, F2], mybir.dt.float32)
            nc.sync.dma_start(b2r_t[:], inp['b2r'][:, :])
            b3r_t = constp.tile([128, F3], mybir.dt.float32)
            nc.sync.dma_start(b3r_t[:], inp['b3r'][:, :])

            h1t = bigp.tile([128, 2, ROWS_PER_CORE], mybir.dt.bfloat16)  # H1.T
            h2t = bigp.tile([F2, ROWS_PER_CORE], mybir.dt.bfloat16)      # H2.T

            # ---- phase 1: M1 = (X @ W1) * c_src, row-chunked AllGather ----
            blocks = [(i * 512, 512) for i in range(12)] + [(6144, 128)]
            for c0, bs in blocks:
                xt = xp.tile([128, KC1, bs], mybir.dt.bfloat16, tag="xt")
                nc.sync.dma_start(
                    xt[:, :, :bs],
                    inp['xT'][:, c0:c0 + bs].rearrange("(kc p) n -> p kc n", p=128))
                for sub in range(bs // 128):
                    t_idx = (c0 + sub * 128) // 128
                    ps = psmm.tile([128, F1], mybir.dt.float32, tag="mm1")
                    for kc in range(KC1):
                        nc.tensor.matmul(ps[:], xt[:, kc, sub * 128:(sub + 1) * 128],
                                         w1_t[:, kc, :], start=(kc == 0), stop=(kc == KC1 - 1))
                    ob = wp.tile([128, F1], mybir.dt.bfloat16, tag="m1o")
                    nc.vector.tensor_scalar(ob[:], ps[:], cs_t[:, t_idx:t_idx + 1], None, AL.mult)
                    m_store(1, t_idx, ob, F1)

            # ---- agg helper: gather + on-chip sel + sel-stationary matmul ----
            def agg_layer(li, elem, fwidth, finish_tile):
                cur = {}
                for call in range(N_CALLS):
                    g = gp.tile([128, CPC, elem], mybir.dt.bfloat16, tag=f"g{elem}")
                    nc.gpsimd.dma_gather(
                        g[:], m_full[li][IDX_OFF:, :],
                        idx_t[:, call * (CALL // 16):(call + 1) * (CALL // 16)],
                        CALL, CALL, elem, queue_num=call % 4)
                    selg = selp.tile([128, CPC, 128], mybir.dt.bfloat16, tag="selg")
                    ch0 = call * CPC
                    nc.vector.tensor_tensor(
                        selg[:], iota_t[:],
                        dstv_t[:, ch0:ch0 + CPC].unsqueeze(2).broadcast_to([128, CPC, 128]),
                        AL.is_equal)
                    for j in range(CPC):
                        ch = ch0 + j
                        t_idx = ch // R_CHUNKS
                        first = (ch % R_CHUNKS == 0)
                        last = (ch % R_CHUNKS == R_CHUNKS - 1)
                        if first:
                            cur[0] = psA.tile([128, fwidth], mybir.dt.float32, tag="aggA", name="psagg")
                        nc.tensor.matmul(cur[0][:], selg[:, j, :], g[:, j, 0:fwidth],
                                         start=first, stop=last)
                        if last:
                            finish_tile(cur[0], t_idx)
                            cur.clear()

            # ---- layer 1 agg -> H1T; fused L2 dense + chunked AG2 ----
            def l1_tile(ps_agg, t_idx):
                sl = slice(t_idx * 128, (t_idx + 1) * 128)
                h1d = wp.tile([128, F1], mybir.dt.bfloat16, tag="h1d")
                nc.vector.scalar_tensor_tensor(
                    h1d[:], ps_agg[:], cdpp_t[:, t_idx:t_idx + 1], b1r_t[:],
                    AL.mult, AL.add)
                nc.scalar.activation(h1d[:], h1d[:], AF.Relu)
                for fc in range(2):
                    trp = psT.tile([128, 128], mybir.dt.bfloat16, tag="tr")
                    nc.tensor.transpose(trp[:], h1d[:, fc * 128:(fc + 1) * 128], ident_t[:])
                    nc.scalar.activation(h1t[:, fc, sl], trp[:], AF.Copy)
                # fused L2 dense for this tile
                ps2 = psmm.tile([128, F1], mybir.dt.float32, tag="mm1", name="ps2")
                for fc in range(2):
                    nc.tensor.matmul(ps2[:, 0:F2], h1t[:, fc, sl], w2_t[:, fc, :],
                                     start=(fc == 0), stop=(fc == 1))
                ob2 = wp.tile([128, F2], mybir.dt.bfloat16, tag="ob2")
                nc.vector.tensor_scalar(ob2[:], ps2[:, 0:F2], cs_t[:, t_idx:t_idx + 1], None, AL.mult)
                m_store(2, t_idx, ob2, F2)

            agg_layer(1, F1, F1, l1_tile)

            # ---- layer 2 agg -> H2T; fused L3 dense + chunked AG3 ----
            def l2_tile(ps_agg, t_idx):
                sl = slice(t_idx * 128, (t_idx + 1) * 128)
                h2d = wp.tile([128, F2], mybir.dt.bfloat16, tag="h2d")
                nc.vector.scalar_tensor_tensor(
                    h2d[:], ps_agg[:], cdpp_t[:, t_idx:t_idx + 1], b2r_t[:],
                    AL.mult, AL.add)
                nc.scalar.activation(h2d[:], h2d[:], AF.Relu)
                trp = psT.tile([128, 128], mybir.dt.bfloat16, tag="tr", name="trp2")
                nc.tensor.transpose(trp[0:F2, :], h2d[:], ident_t[:])
                nc.scalar.activation(h2t[:, sl], trp[0:F2, :], AF.Copy)
                # fused L3 dense for this tile
                ps3 = psmm.tile([128, F1], mybir.dt.float32, tag="mm1", name="ps3")
                nc.tensor.matmul(ps3[:, 0:F3], h2t[:, sl], w3_t[:], start=True, stop=True)
                ob3 = wp.tile([128, F3], mybir.dt.bfloat16, tag="ob3")
                nc.vector.tensor_scalar(ob3[:], ps3[:, 0:F3], cs_t[:, t_idx:t_idx + 1], None, AL.mult)
                m_store(3, t_idx, ob3, F3)

            agg_layer(2, FPAD, F2, l2_tile)

            # ---- layer 3 agg -> logits; softmax batched in two halves ----
            xall = bigp.tile([128, TILES_PER_CORE * F3], mybir.dt.float32)

            def softmax_part(tlo, thi):
                flo, fhi = tlo * F3, thi * F3
                exa = wp.tile([128, (thi - tlo) * F3], mybir.dt.float32,
                              tag="exa", name="exa")
                nc.scalar.activation(exa[:], xall[:, flo:fhi], AF.Exp)
                smv = wp.tile([128, thi - tlo], mybir.dt.float32, tag="smv", name="smv")
                nc.vector.tensor_reduce(
                    smv[:], exa[:].rearrange("p (t f) -> p t f", f=F3),
                    mybir.AxisListType.X, AL.add)
                rsv = wp.tile([128, thi - tlo], mybir.dt.float32, tag="rsv", name="rsv")
                nc.vector.reciprocal(rsv[:], smv[:])
                nlog = wp.tile([128, thi - tlo], mybir.dt.float32, tag="nlog", name="nlog")
                nc.scalar.activation(nlog[:], rsv[:], AF.Ln)
                for t_idx in range(tlo, thi):
                    sl3 = slice(t_idx * F3, (t_idx + 1) * F3)
                    ox = wp.tile([128, F3], mybir.dt.float32, tag="ox")
                    nc.vector.tensor_scalar(ox[:], xall[:, sl3],
                                            nlog[:, t_idx - tlo:t_idx - tlo + 1], None, AL.add)
                    nc.sync.dma_start(out_t[t_idx * 128:(t_idx + 1) * 128, :], ox[:])

            def l3_tile(ps_agg, t_idx):
                sl3 = slice(t_idx * F3, (t_idx + 1) * F3)
                nc.vector.scalar_tensor_tensor(
                    xall[:, sl3], ps_agg[:], cdpp_t[:, t_idx:t_idx + 1], b3r_t[:],
                    AL.mult, AL.add)
                if t_idx == 23:
                    softmax_part(0, 24)

            agg_layer(3, FPAD, F3, l3_tile)
            softmax_part(24, TILES_PER_CORE)

    nc.compile()
    return nc


def _install_profile_shim():
    """Provide the missing antenv.axon_hooks module so trace=True works under axon."""
    try:
        import types
        import antenv
        if 'antenv.axon_hooks' in sys.modules:
            return
        _hook = [None]
        mod = types.ModuleType('antenv.axon_hooks')
        mod.set_axon_ntff_profile_hook = lambda h: _hook.__setitem__(0, h)
        mod.get_axon_ntff_profile_hook = lambda: _hook[0]
        sys.modules['antenv.axon_hooks'] = mod
        antenv.axon_hooks = mod
        from trn_agent_boot.trn_boot import _ntff_profile_via_ctypes
        mod.set_axon_ntff_profile_hook(
            _ntff_profile_via_ctypes('/opt/axon/libaxon_pjrt.so'))
    except Exception:
        pass


_CACHE = {}


def kernel(features, edge_index, W1, b1, W2, b2, W3, b3):
    global last_exec_time_ns
    features = np.asarray(features, dtype=np.float32)
    pre = _preprocess(np.asarray(edge_index))

    if 'nc' not in _CACHE:
        _CACHE['nc'] = _build_nc()
    nc = _CACHE['nc']

    # host-side input prep
    W1p = np.zeros((F_IN_P, F1), dtype=BF16)
    W1p[:F_IN] = np.asarray(W1, dtype=BF16)
    W2b = np.asarray(W2, dtype=BF16)
    W3b = np.asarray(W3, dtype=BF16)
    b1r = np.tile(np.asarray(b1, dtype=np.float32), (128, 1))
    b2r = np.tile(np.asarray(b2, dtype=np.float32), (128, 1))
    b3r = np.tile(np.asarray(b3, dtype=np.float32), (128, 1))
    iota = np.ascontiguousarray(np.broadcast_to(
        np.arange(128, dtype=np.float32), (128, CPC, 128))).astype(BF16)
    ident = np.eye(128, dtype=BF16)

    # features, permuted and transposed per core: [F_IN_P, 6272] bf16
    feat_b = features.astype(BF16)
    in_maps = []
    for c in range(N_CORES):
        rows = pre['node_of_row'][c * ROWS_PER_CORE:(c + 1) * ROWS_PER_CORE]
        xTc = np.zeros((F_IN_P, ROWS_PER_CORE), dtype=BF16)
        real = rows >= 0
        xTc[:F_IN, real] = feat_b[rows[real]].T
        in_maps.append({
            'xT': xTc, 'W1': W1p, 'W2': W2b, 'W3': W3b,
            'idx': pre['idx_tile'][c], 'dstv': pre['dstv'][c],
            'iota': iota, 'ident': ident,
            'cdst_pp': pre['cdst_pp'][c], 'csrc_t': pre['csrc_t'][c],
            'b1r': b1r, 'b2r': b2r, 'b3r': b3r,
        })

    trace = os.environ.get('BASS_KERNEL_TRACE', '0') == '1'
    if trace:
        _install_profile_shim()
    res = run_bass_kernel_spmd(nc, in_maps, core_ids=list(range(N_CORES)), trace=trace)
    last_exec_time_ns = res.exec_time_ns

    # assemble + inverse permute
    out_rows = np.concatenate([res.results[c]['out'] for c in range(N_CORES)], axis=0)
    out = np.empty((N_NODES, F3), dtype=np.float32)
    real = pre['node_of_row'] >= 0
    out[pre['node_of_row'][real]] = out_rows[real]
    return out


# revision 15
# speedup vs baseline: 1.6352x; 1.1987x over previous
"""Trainium2 Bass kernel for nn_DGL_Net (3-layer GraphConv GNN, 50000 nodes, 800k edges).

Strategy (8 NeuronCores, SPMD):
  - Host: relabel nodes into 392 balanced tiles of 128 nodes (<=2046 in-edges per
    tile), 49 tiles per core. Per layer: local matmul (bf16) -> scale by c_src ->
    AllGather (row-chunked, overlapped) -> per-edge dma_gather (4 SWDGE queues,
    src-sorted within each tile for HBM locality) -> one-hot (Sel) matmul
    aggregation in PSUM -> scale by c_dst + bias (+relu / log_softmax).
  - Sel one-hot matrices are generated ON-CHIP (one DVE is_equal per 1024-edge
    gather call, comparing a resident iota tile against broadcast dst lanes)
    instead of streaming 25.7MB/layer of precomputed one-hots from HBM.
  - Aggregation matmul is sel-stationary: PSUM[d,f] += Sel[e,d].T @ G[e,f] per
    128-edge chunk (one matmul per chunk). Per-tile epilogue applies
    c_dst (per-partition scalar) + bias + relu, PE-transposes back to [f,d]
    layout, and immediately runs the NEXT layer's dense matmul for that tile
    so the AllGathers can start early (chunked, overlapped with compute).
  - int16 gather indices: gather base is offset +32768 rows so idx = row-32768
    spans the whole [0, 50176) row space within int16. The last slot of every
    1024-index gather call is a reserved dummy with idx>=0 (defeats the ucode's
    trailing-negative trim).
"""
import os
import sys

sys.path.insert(0, '/opt/trn_rl_repo')

import numpy as np
import ml_dtypes

import concourse.bass as bass
import concourse.bacc as bacc
import concourse.mybir as mybir
import concourse.tile as tile
from concourse.bass_utils import run_bass_kernel_spmd

BF16 = ml_dtypes.bfloat16

N_NODES = 50000
N_CORES = 8
TILE_N = 128                 # nodes per tile
TILES_PER_CORE = 49
N_TILES = N_CORES * TILES_PER_CORE      # 392
ROWS_PER_CORE = TILES_PER_CORE * TILE_N  # 6272
N_ROWS = N_CORES * ROWS_PER_CORE         # 50176
R_CHUNKS = 16                # edge chunks (of 128 slots) per tile
SLOTS_PER_TILE = R_CHUNKS * 128          # 2048
TILE_EDGE_CAP = SLOTS_PER_TILE - 2       # 2046 (2 reserved call-end dummies)
SLOTS = TILES_PER_CORE * SLOTS_PER_TILE  # 100352 per core
CALL = 1024                  # idxs per dma_gather call
CPC = CALL // 128            # chunks per call (8)
N_CALLS = SLOTS // CALL      # 98
CHUNKS = TILES_PER_CORE * R_CHUNKS       # 784 chunks per core
IDX_OFF = 32768              # gather base offset (int16 trick)
F_IN = 1433
F_IN_P = 1536                # padded to 12*128
KC1 = F_IN_P // 128          # 12
F1 = 256
F2 = 32
F3 = 7
FPAD = 128                   # padded row width for M2/M3 gather (256B elems)
# AllGather row segments (tile counts); fired as each segment's tiles finish.
# The last segment is a single tile so the final (exposed) AllGather is tiny.
SEG_T = [16, 16, 15, 2]
SEG_R = [t * TILE_N for t in SEG_T]               # [2048, 2048, 1920, 256]
SEG_START = [0, 2048, 4096, 6016]                 # per-core local row starts
SEG_FULL = [0, 16384, 32768, 48128]               # chunk-major full-table starts
AG_FIRE = {15: 0, 31: 1, 46: 2, 48: 3}            # t_idx -> segment to fire

last_exec_time_ns = None


def _preprocess(edge_index):
    """Graph preprocessing: normalization constants, node->($core,tile,lane)
    relabeling with balanced per-tile in-degree, per-core edge slot tables
    (slots sorted by src row within each tile for gather locality)."""
    src = np.asarray(edge_index[0], dtype=np.int64)
    dst = np.asarray(edge_index[1], dtype=np.int64)
    n_edges = src.shape[0]

    deg_out = np.bincount(src, minlength=N_NODES).astype(np.float64)
    deg_in = np.bincount(dst, minlength=N_NODES).astype(np.float64)
    c_src = (1.0 / np.sqrt(np.maximum(deg_out, 1.0))).astype(np.float32)
    c_dst = (1.0 / np.sqrt(np.maximum(deg_in, 1.0))).astype(np.float32)

    # --- greedy balanced tile packing by in-degree ---
    import heapq
    order = np.argsort(-deg_in, kind='stable')
    heap = [(0.0, 0, t) for t in range(N_TILES)]  # (load, count, tile)
    heapq.heapify(heap)
    tile_nodes = [[] for _ in range(N_TILES)]
    tile_load = np.zeros(N_TILES)
    deferred = []
    for v in order:
        dv = deg_in[v]
        while True:
            load, cnt, t = heapq.heappop(heap)
            if cnt >= TILE_N:
                continue  # stale/full
            if load + dv > TILE_EDGE_CAP:
                deferred.append((load, cnt, t))
                continue
            break
        tile_nodes[t].append(int(v))
        tile_load[t] = load + dv
        heapq.heappush(heap, (load + dv, cnt + 1, t))
        for item in deferred:
            heapq.heappush(heap, item)
        deferred = []
    assert max(tile_load) <= TILE_EDGE_CAP

    # sort tiles by load desc, group by 8, core c takes c-th of each group
    tsort = np.argsort(-tile_load, kind='stable')
    tile_assign = np.empty((N_CORES, TILES_PER_CORE), dtype=np.int64)
    for k in range(TILES_PER_CORE):
        for c in range(N_CORES):
            tile_assign[c, k] = tsort[k * N_CORES + c]

    # row mapping: row = c*ROWS_PER_CORE + k*128 + lane
    row_of_node = np.full(N_NODES, -1, dtype=np.int64)
    node_of_row = np.full(N_ROWS, -1, dtype=np.int64)  # -1 = virtual pad node
    for c in range(N_CORES):
        for k in range(TILES_PER_CORE):
            t = tile_assign[c, k]
            nodes = tile_nodes[t]
            base = c * ROWS_PER_CORE + k * TILE_N
            for lane, v in enumerate(nodes):
                row_of_node[v] = base + lane
                node_of_row[base + lane] = v
    assert (row_of_node >= 0).all()

    # --- per-core edge slot tables ---
    dst_row = row_of_node[dst]
    src_row = row_of_node[src]
    e_core = dst_row // ROWS_PER_CORE
    e_tile = (dst_row % ROWS_PER_CORE) // TILE_N   # k within core
    e_lane = dst_row % TILE_N

    idx_flat = np.zeros((N_CORES, SLOTS), dtype=np.int16)      # pad idx = 0
    dst_flat = np.full((N_CORES, SLOTS), -1, dtype=np.int16)   # pad dst = -1

    # gather-row renumbering: full tables are laid out chunk-major
    # ([all cores' seg-0 rows, then seg-1, ...]) so each chunked AllGather
    # output is contiguous
    seg_start = np.asarray(SEG_START)
    seg_r = np.asarray(SEG_R)
    seg_full = np.asarray(SEG_FULL)
    sc = src_row // ROWS_PER_CORE
    sr = src_row % ROWS_PER_CORE
    seg_i = np.searchsorted(seg_start, sr, side='right') - 1
    src_grow = seg_full[seg_i] + sc * seg_r[seg_i] + (sr - seg_start[seg_i])

    # group edges by (core, tile); within each tile sort by src row (gather
    # locality), then assign slot positions skipping reserved slots 1023/2047
    key = e_core * TILES_PER_CORE + e_tile
    eorder = np.lexsort((src_grow, key))   # sort by key, then src gather-row
    key_s = key[eorder]
    grp_start = np.searchsorted(key_s, np.arange(N_CORES * TILES_PER_CORE))
    pos_in_grp = np.arange(n_edges) - grp_start[key_s]
    j = pos_in_grp
    slot_in_tile = j + (j >= 1023).astype(np.int64)  # j>=1023 shifts past slot 1023
    assert slot_in_tile.max() < SLOTS_PER_TILE - 1   # never hits 2047
    slots_abs = key_s % TILES_PER_CORE * SLOTS_PER_TILE + slot_in_tile
    cores_s = key_s // TILES_PER_CORE
    idx_flat[cores_s, slots_abs] = (src_grow[eorder] - IDX_OFF).astype(np.int16)
    dst_flat[cores_s, slots_abs] = e_lane[eorder].astype(np.int16)

    # wrap idx to [128, SLOTS/16] (idx i -> [i%16 replicated, i//16])
    cols = SLOTS // 16
    idx_tile = np.zeros((N_CORES, 128, cols), dtype=np.int16)
    for c in range(N_CORES):
        w = idx_flat[c].reshape(cols, 16).T  # [16, cols]
        idx_tile[c] = np.tile(w, (8, 1))

    # dst lane per slot, wrapped [128 lanes, CHUNKS] bf16 (for on-chip sel-gen)
    dstv = np.empty((N_CORES, 128, CHUNKS), dtype=BF16)
    for c in range(N_CORES):
        dstv[c] = dst_flat[c].reshape(CHUNKS, 128).T.astype(BF16)

    # per-core normalization tables
    cd_row = np.where(node_of_row >= 0, c_dst[np.maximum(node_of_row, 0)], 1.0)
    cs_row = np.where(node_of_row >= 0, c_src[np.maximum(node_of_row, 0)], 1.0)
    cd_core = cd_row.reshape(N_CORES, ROWS_PER_CORE).astype(np.float32)
    cs_core = cs_row.reshape(N_CORES, ROWS_PER_CORE).astype(np.float32)
    cdst_pp = cd_core.reshape(N_CORES, TILES_PER_CORE, 128).transpose(0, 2, 1).copy()
    csrc_t = cs_core.reshape(N_CORES, TILES_PER_CORE, 128).transpose(0, 2, 1).copy()

    return dict(row_of_node=row_of_node, node_of_row=node_of_row,
                idx_tile=idx_tile, dstv=dstv,
                cdst_pp=cdst_pp, csrc_t=csrc_t)


def _build_nc():
    nc = bacc.Bacc("TRN2", target_bir_lowering=False, debug=False,
                   enable_asserts=True, num_devices=N_CORES, num_swdge_queues=4)
    dt = mybir.dt
    inp = {}
    inp['xT'] = nc.dram_tensor("xT", [F_IN_P, ROWS_PER_CORE], dt.bfloat16, kind="ExternalInput")
    inp['W1'] = nc.dram_tensor("W1", [F_IN_P, F1], dt.bfloat16, kind="ExternalInput")
    inp['W2'] = nc.dram_tensor("W2", [F1, F2], dt.bfloat16, kind="ExternalInput")
    inp['W3'] = nc.dram_tensor("W3", [F2, F3], dt.bfloat16, kind="ExternalInput")
    inp['idx'] = nc.dram_tensor("idx", [128, SLOTS // 16], dt.int16, kind="ExternalInput")
    inp['dstv'] = nc.dram_tensor("dstv", [128, CHUNKS], dt.bfloat16, kind="ExternalInput")
    inp['iota'] = nc.dram_tensor("iota", [128, CPC, 128], dt.bfloat16, kind="ExternalInput")
    inp['ident'] = nc.dram_tensor("ident", [128, 128], dt.bfloat16, kind="ExternalInput")
    inp['cdst_pp'] = nc.dram_tensor("cdst_pp", [128, TILES_PER_CORE], dt.float32, kind="ExternalInput")
    inp['csrc_t'] = nc.dram_tensor("csrc_t", [128, TILES_PER_CORE], dt.float32, kind="ExternalInput")
    inp['b1r'] = nc.dram_tensor("b1r", [128, F1], dt.float32, kind="ExternalInput")
    inp['b2r'] = nc.dram_tensor("b2r", [128, F2], dt.float32, kind="ExternalInput")
    inp['b3r'] = nc.dram_tensor("b3r", [128, F3], dt.float32, kind="ExternalInput")
    out_t = nc.dram_tensor("out", [ROWS_PER_CORE, F3], dt.float32, kind="ExternalOutput")

    m_own = {}
    for li, w in ((1, F1), (2, FPAD), (3, FPAD)):
        for sg in range(4):
            m_own[li, sg] = nc.dram_tensor(f"m{li}_own_{sg}", [SEG_R[sg], w], dt.bfloat16)
    m_full = {
        1: nc.dram_tensor("m1_full", [N_ROWS, F1], dt.bfloat16, addr_space="Shared"),
        2: nc.dram_tensor("m2_full", [N_ROWS, FPAD], dt.bfloat16, addr_space="Shared"),
        3: nc.dram_tensor("m3_full", [N_ROWS, FPAD], dt.bfloat16, addr_space="Shared"),
    }

    AL = mybir.AluOpType
    AF = mybir.ActivationFunctionType
    RG = [list(range(N_CORES))]

    def ag_seg(li, sg):
        """AllGather segment sg of layer li's table (contiguous chunk-major rows)."""
        lo = SEG_FULL[sg]
        hi = lo + N_CORES * SEG_R[sg]
        nc.gpsimd.collective_compute(
            "AllGather", AL.bypass, replica_groups=RG,
            ins=[m_own[li, sg][:, :]], outs=[m_full[li][lo:hi, :]])

    def m_store(li, t_idx, ob, w):
        sg = int(np.searchsorted(np.asarray(SEG_START), t_idx * 128, side='right')) - 1
        r = t_idx * 128 - SEG_START[sg]
        nc.sync.dma_start(m_own[li, sg][r:r + 128, 0:w], ob[:])
        if t_idx in AG_FIRE:
            ag_seg(li, AG_FIRE[t_idx])

    with tile.TileContext(nc) as tc:
        with tc.tile_pool(name="const", bufs=1) as constp, \
             tc.tile_pool(name="big", bufs=1) as bigp, \
             tc.tile_pool(name="xstream", bufs=2) as xp, \
             tc.tile_pool(name="work", bufs=4) as wp, \
             tc.tile_pool(name="gpool", bufs=6) as gp, \
             tc.tile_pool(name="selp", bufs=6) as selp, \
             tc.tile_pool(name="psA", bufs=2, space="PSUM") as psA, \
             tc.tile_pool(name="psT", bufs=2, space="PSUM") as psT, \
             tc.tile_pool(name="psmm", bufs=2, space="PSUM") as psmm:

            # ---- resident constants ----
            w1_t = constp.tile([128, KC1, F1], mybir.dt.bfloat16)
            nc.sync.dma_start(w1_t[:], inp['W1'].rearrange("(kc p) n -> p kc n", p=128))
            w2_t = constp.tile([128, 2, F2], mybir.dt.bfloat16)
            nc.sync.dma_start(w2_t[:], inp['W2'].rearrange("(kc p) n -> p kc n", p=128))
            w3_t = constp.tile([F2, F3], mybir.dt.bfloat16)
            nc.sync.dma_start(w3_t[:], inp['W3'][:, :])
            idx_t = constp.tile([128, SLOTS // 16], mybir.dt.int16)
            nc.sync.dma_start(idx_t[:], inp['idx'][:, :])
            dstv_t = constp.tile([128, CHUNKS], mybir.dt.bfloat16)
            nc.sync.dma_start(dstv_t[:], inp['dstv'][:, :])
            iota_t = constp.tile([128, CPC, 128], mybir.dt.bfloat16)
            nc.sync.dma_start(iota_t[:], inp['iota'][:, :, :])
            ident_t = constp.tile([128, 128], mybir.dt.bfloat16)
            nc.sync.dma_start(ident_t[:], inp['ident'][:, :])
            cdpp_t = constp.tile([128, TILES_PER_CORE], mybir.dt.float32)
            nc.sync.dma_start(cdpp_t[:], inp['cdst_pp'][:, :])
            cs_t = constp.tile([128, TILES_PER_CORE], mybir.dt.float32)
            nc.sync.dma_start(cs_t[:], inp['csrc_t'][:, :])
            b1r_t = constp.tile([128, F1], mybir.dt.float32)
            nc.sync.dma_start(b1r_t[:], inp['b1r'][:, :])
            b2r_t = constp.tile([128, F2], mybir.dt.float32)
            nc.sync.dma_start(b2r_t[:], inp['b2r'][:, :])
            b3r_t = constp.tile([128, F3], mybir.dt.float32)
            nc.sync.dma_start(b3r_t[:], inp['b3r'][:, :])

            h1t = bigp.tile([128, 2, ROWS_PER_CORE], mybir.dt.bfloat16)  # H1.T
            h2t = bigp.tile([F2, ROWS_PER_CORE], mybir.dt.bfloat16)      # H2.T

            # ---- phase 1: M1 = (X @ W1) * c_src, row-chunked AllGather ----
            blocks = [(i * 512, 512) for i in range(12)] + [(6144, 128)]
            for c0, bs in blocks:
                xt = xp.tile([128, KC1, bs], mybir.dt.bfloat16, tag="xt")
                nc.sync.dma_start(
                    xt[:, :, :bs],
                    inp['xT'][:, c0:c0 + bs].rearrange("(kc p) n -> p kc n", p=128))
                for sub in range(bs // 128):
                    t_idx = (c0 + sub * 128) // 128
                    ps = psmm.tile([128, F1], mybir.dt.float32, tag="mm1")
                    for kc in range(KC1):
                        nc.tensor.matmul(ps[:], xt[:, kc, sub * 128:(sub + 1) * 128],
                                         w1_t[:, kc, :], start=(kc == 0), stop=(kc == KC1 - 1))
                    ob = wp.tile([128, F1], mybir.dt.bfloat16, tag="m1o")
                    nc.vector.tensor_scalar(ob[:], ps[:], cs_t[:, t_idx:t_idx + 1], None, AL.mult)
                    m_store(1, t_idx, ob, F1)

            # ---- agg helper: gather + on-chip sel + sel-stationary matmul ----
            def agg_layer(li, elem, fwidth, finish_tile):
                cur = {}
                for call in range(N_CALLS):
                    g = gp.tile([128, CPC, elem], mybir.dt.bfloat16, tag=f"g{elem}")
                    nc.gpsimd.dma_gather(
                        g[:], m_full[li][IDX_OFF:, :],
                        idx_t[:, call * (CALL // 16):(call + 1) * (CALL // 16)],
                        CALL, CALL, elem, queue_num=call % 4)
                    selg = selp.tile([128, CPC, 128], mybir.dt.bfloat16, tag="selg")
                    ch0 = call * CPC
                    nc.vector.tensor_tensor(
                        selg[:], iota_t[:],
                        dstv_t[:, ch0:ch0 + CPC].unsqueeze(2).broadcast_to([128, CPC, 128]),
                        AL.is_equal)
                    for j in range(CPC):
                        ch = ch0 + j
                        t_idx = ch // R_CHUNKS
                        first = (ch % R_CHUNKS == 0)
                        last = (ch % R_CHUNKS == R_CHUNKS - 1)
                        if first:
                            cur[0] = psA.tile([128, fwidth], mybir.dt.float32, tag="aggA", name="psagg")
                        nc.tensor.matmul(cur[0][:], selg[:, j, :], g[:, j, 0:fwidth],
                                         start=first, stop=last)
                        if last:
                            finish_tile(cur[0], t_idx)
                            cur.clear()

            # ---- layer 1 agg -> H1T; fused L2 dense + chunked AG2 ----
            def l1_tile(ps_agg, t_idx):
                sl = slice(t_idx * 128, (t_idx + 1) * 128)
                h1d = wp.tile([128, F1], mybir.dt.bfloat16, tag="h1d")
                nc.vector.scalar_tensor_tensor(
                    h1d[:], ps_agg[:], cdpp_t[:, t_idx:t_idx + 1], b1r_t[:],
                    AL.mult, AL.add)
                nc.scalar.activation(h1d[:], h1d[:], AF.Relu)
                for fc in range(2):
                    trp = psT.tile([128, 128], mybir.dt.bfloat16, tag="tr")
                    nc.tensor.transpose(trp[:], h1d[:, fc * 128:(fc + 1) * 128], ident_t[:])
                    nc.scalar.activation(h1t[:, fc, sl], trp[:], AF.Copy)
                # fused L2 dense for this tile
                ps2 = psmm.tile([128, F1], mybir.dt.float32, tag="mm1", name="ps2")
                for fc in range(2):
                    nc.tensor.matmul(ps2[:, 0:F2], h1t[:, fc, sl], w2_t[:, fc, :],
                                     start=(fc == 0), stop=(fc == 1))
                ob2 = wp.tile([128, F2], mybir.dt.bfloat16, tag="ob2")
                nc.vector.tensor_scalar(ob2[:], ps2[:, 0:F2], cs_t[:, t_idx:t_idx + 1], None, AL.mult)
                m_store(2, t_idx, ob2, F2)

            agg_layer(1, F1, F1, l1_tile)

            # ---- layer 2 agg -> H2T; fused L3 dense + chunked AG3 ----
            def l2_tile(ps_agg, t_idx):
                sl = slice(t_idx * 128, (t_idx + 1) * 128)
                h2d = wp.tile([128, F2], mybir.dt.bfloat16, tag="h2d")
                nc.vector.scalar_tensor_tensor(
                    h2d[:], ps_agg[:], cdpp_t[:, t_idx:t_idx + 1], b2r_t[:],
                    AL.mult, AL.add)
                nc.scalar.activation(h2d[:], h2d[:], AF.Relu)
                trp = psT.tile([128, 128], mybir.dt.bfloat16, tag="tr", name="trp2")
                nc.tensor.transpose(trp[0:F2, :], h2d[:], ident_t[:])
                nc.scalar.activation(h2t[:, sl], trp[0:F2, :], AF.Copy)
                # fused L3 dense for this tile
                ps3 = psmm.tile([128, F1], mybir.dt.float32, tag="mm1", name="ps3")
                nc.tensor.matmul(ps3[:, 0:F3], h2t[:, sl], w3_t[:], start=True, stop=True)
                ob3 = wp.tile([128, F3], mybir.dt.bfloat16, tag="ob3")
                nc.vector.tensor_scalar(ob3[:], ps3[:, 0:F3], cs_t[:, t_idx:t_idx + 1], None, AL.mult)
                m_store(3, t_idx, ob3, F3)

            agg_layer(2, FPAD, F2, l2_tile)

            # ---- layer 3 agg -> logits; softmax batched in two halves ----
            xall = bigp.tile([128, TILES_PER_CORE * F3], mybir.dt.float32)

            def softmax_part(tlo, thi):
                flo, fhi = tlo * F3, thi * F3
                exa = wp.tile([128, (thi - tlo) * F3], mybir.dt.float32,
                              tag="exa", name="exa")
                nc.scalar.activation(exa[:], xall[:, flo:fhi], AF.Exp)
                smv = wp.tile([128, thi - tlo], mybir.dt.float32, tag="smv", name="smv")
                nc.vector.tensor_reduce(
                    smv[:], exa[:].rearrange("p (t f) -> p t f", f=F3),
                    mybir.AxisListType.X, AL.add)
                rsv = wp.tile([128, thi - tlo], mybir.dt.float32, tag="rsv", name="rsv")
                nc.vector.reciprocal(rsv[:], smv[:])
                nlog = wp.tile([128, thi - tlo], mybir.dt.float32, tag="nlog", name="nlog")
                nc.scalar.activation(nlog[:], rsv[:], AF.Ln)
                for t_idx in range(tlo, thi):
                    sl3 = slice(t_idx * F3, (t_idx + 1) * F3)
                    ox = wp.tile([128, F3], mybir.dt.float32, tag="ox")
                    nc.vector.tensor_scalar(ox[:], xall[:, sl3],
                                            nlog[:, t_idx - tlo:t_idx - tlo + 1], None, AL.add)
                    nc.sync.dma_start(out_t[t_idx * 128:(t_idx + 1) * 128, :], ox[:])

            def l3_tile(ps_agg, t_idx):
                sl3 = slice(t_idx * F3, (t_idx + 1) * F3)
                nc.vector.scalar_tensor_tensor(
                    xall[:, sl3], ps_agg[:], cdpp_t[:, t_idx:t_idx + 1], b3r_t[:],
                    AL.mult, AL.add)
                if t_idx == 23:
                    softmax_part(0, 24)

            agg_layer(3, FPAD, F3, l3_tile)
            softmax_part(24, TILES_PER_CORE)

    nc.compile()
    return nc


def _install_profile_shim():
    """Provide the missing antenv.axon_hooks module so trace=True works under axon."""
    try:
        import types
        import antenv
        if 'antenv.axon_hooks' in sys.modules:
            return
        _hook = [None]
        mod = types.ModuleType('antenv.axon_hooks')
        mod.set_axon_ntff_profile_hook = lambda h: _hook.__setitem__(0, h)
        mod.get_axon_ntff_profile_hook = lambda: _hook[0]
        sys.modules['antenv.axon_hooks'] = mod
        antenv.axon_hooks = mod
        from trn_agent_boot.trn_boot import _ntff_profile_via_ctypes
        mod.set_axon_ntff_profile_hook(
            _ntff_profile_via_ctypes('/opt/axon/libaxon_pjrt.so'))
    except Exception:
        pass


_CACHE = {}


def kernel(features, edge_index, W1, b1, W2, b2, W3, b3):
    global last_exec_time_ns
    features = np.asarray(features, dtype=np.float32)
    pre = _preprocess(np.asarray(edge_index))

    if 'nc' not in _CACHE:
        _CACHE['nc'] = _build_nc()
    nc = _CACHE['nc']

    # host-side input prep
    W1p = np.zeros((F_IN_P, F1), dtype=BF16)
    W1p[:F_IN] = np.asarray(W1, dtype=BF16)
    W2b = np.asarray(W2, dtype=BF16)
    W3b = np.asarray(W3, dtype=BF16)
    b1r = np.tile(np.asarray(b1, dtype=np.float32), (128, 1))
    b2r = np.tile(np.asarray(b2, dtype=np.float32), (128, 1))
    b3r = np.tile(np.asarray(b3, dtype=np.float32), (128, 1))
    iota = np.ascontiguousarray(np.broadcast_to(
        np.arange(128, dtype=np.float32), (128, CPC, 128))).astype(BF16)
    ident = np.eye(128, dtype=BF16)

    # features, permuted and transposed per core: [F_IN_P, 6272] bf16
    feat_b = features.astype(BF16)
    in_maps = []
    for c in range(N_CORES):
        rows = pre['node_of_row'][c * ROWS_PER_CORE:(c + 1) * ROWS_PER_CORE]
        xTc = np.zeros((F_IN_P, ROWS_PER_CORE), dtype=BF16)
        real = rows >= 0
        xTc[:F_IN, real] = feat_b[rows[real]].T
        in_maps.append({
            'xT': xTc, 'W1': W1p, 'W2': W2b, 'W3': W3b,
            'idx': pre['idx_tile'][c], 'dstv': pre['dstv'][c],
            'iota': iota, 'ident': ident,
            'cdst_pp': pre['cdst_pp'][c], 'csrc_t': pre['csrc_t'][c],
            'b1r': b1r, 'b2r': b2r, 'b3r': b3r,
        })

    trace = os.environ.get('BASS_KERNEL_TRACE', '0') == '1'
    if trace:
        _install_profile_shim()
    res = run_bass_kernel_spmd(nc, in_maps, core_ids=list(range(N_CORES)), trace=trace)
    last_exec_time_ns = res.exec_time_ns

    # assemble + inverse permute
    out_rows = np.concatenate([res.results[c]['out'] for c in range(N_CORES)], axis=0)
    out = np.empty((N_NODES, F3), dtype=np.float32)
    real = pre['node_of_row'] >= 0
    out[pre['node_of_row'][real]] = out_rows[real]
    return out


# revision 16
# speedup vs baseline: 1.7119x; 1.0469x over previous
"""Trainium2 Bass kernel for nn_DGL_Net (3-layer GraphConv GNN, 50000 nodes, 800k edges).

Strategy (8 NeuronCores, SPMD):
  - Host: relabel nodes into 392 balanced tiles of 128 nodes (<=2046 in-edges per
    tile), 49 tiles per core. Per layer: local matmul (bf16) -> scale by c_src ->
    AllGather (row-chunked, overlapped) -> per-edge dma_gather (4 SWDGE queues,
    src-sorted within each tile for HBM locality) -> one-hot (Sel) matmul
    aggregation in PSUM -> scale by c_dst + bias (+relu / log_softmax).
  - Sel one-hot matrices are generated ON-CHIP (one DVE is_equal per 1024-edge
    gather call, comparing a resident iota tile against broadcast dst lanes)
    instead of streaming 25.7MB/layer of precomputed one-hots from HBM.
  - Aggregation matmul is sel-stationary: PSUM[d,f] += Sel[e,d].T @ G[e,f] per
    128-edge chunk (one matmul per chunk). Per-tile epilogue applies
    c_dst (per-partition scalar) + bias + relu, PE-transposes back to [f,d]
    layout, and immediately runs the NEXT layer's dense matmul for that tile
    so the AllGathers can start early (chunked, overlapped with compute).
  - int16 gather indices: gather base is offset +32768 rows so idx = row-32768
    spans the whole [0, 50176) row space within int16. The last slot of every
    1024-index gather call is a reserved dummy with idx>=0 (defeats the ucode's
    trailing-negative trim).
"""
import os
import sys

sys.path.insert(0, '/opt/trn_rl_repo')

import numpy as np
import ml_dtypes

import concourse.bass as bass
import concourse.bacc as bacc
import concourse.mybir as mybir
import concourse.tile as tile
from concourse.bass_utils import run_bass_kernel_spmd

BF16 = ml_dtypes.bfloat16

N_NODES = 50000
N_CORES = 8
TILE_N = 128                 # nodes per tile
TILES_PER_CORE = 49
N_TILES = N_CORES * TILES_PER_CORE      # 392
ROWS_PER_CORE = TILES_PER_CORE * TILE_N  # 6272
N_ROWS = N_CORES * ROWS_PER_CORE         # 50176
R_CHUNKS = 16                # edge chunks (of 128 slots) per tile
SLOTS_PER_TILE = R_CHUNKS * 128          # 2048
TILE_EDGE_CAP = SLOTS_PER_TILE - 2       # 2046 (2 reserved call-end dummies)
SLOTS = TILES_PER_CORE * SLOTS_PER_TILE  # 100352 per core
CALL = 1024                  # idxs per dma_gather call
CPC = CALL // 128            # chunks per call (8)
N_CALLS = SLOTS // CALL      # 98
CHUNKS = TILES_PER_CORE * R_CHUNKS       # 784 chunks per core
IDX_OFF = 32768              # gather base offset (int16 trick)
F_IN = 1433
F_IN_P = 1536                # padded to 12*128
KC1 = F_IN_P // 128          # 12
F1 = 256
F2 = 32
F3 = 7
FPAD = 128                   # padded row width for M2/M3 gather (256B elems)
# AllGather row segments (tile counts); fired as each segment's tiles finish.
# The last segment is a single tile so the final (exposed) AllGather is tiny.
SEG_T = [16, 16, 15, 2]
SEG_R = [t * TILE_N for t in SEG_T]               # [2048, 2048, 1920, 256]
SEG_START = [0, 2048, 4096, 6016]                 # per-core local row starts
SEG_FULL = [0, 16384, 32768, 48128]               # chunk-major full-table starts
AG_FIRE = {15: 0, 31: 1, 46: 2, 48: 3}            # t_idx -> segment to fire

last_exec_time_ns = None


def _preprocess(edge_index):
    """Graph preprocessing: normalization constants, node->($core,tile,lane)
    relabeling with balanced per-tile in-degree, per-core edge slot tables
    (slots sorted by src row within each tile for gather locality)."""
    src = np.asarray(edge_index[0], dtype=np.int64)
    dst = np.asarray(edge_index[1], dtype=np.int64)
    n_edges = src.shape[0]

    deg_out = np.bincount(src, minlength=N_NODES).astype(np.float64)
    deg_in = np.bincount(dst, minlength=N_NODES).astype(np.float64)
    c_src = (1.0 / np.sqrt(np.maximum(deg_out, 1.0))).astype(np.float32)
    c_dst = (1.0 / np.sqrt(np.maximum(deg_in, 1.0))).astype(np.float32)

    # --- greedy balanced tile packing by in-degree ---
    import heapq
    order = np.argsort(-deg_in, kind='stable')
    heap = [(0.0, 0, t) for t in range(N_TILES)]  # (load, count, tile)
    heapq.heapify(heap)
    tile_nodes = [[] for _ in range(N_TILES)]
    tile_load = np.zeros(N_TILES)
    deferred = []
    for v in order:
        dv = deg_in[v]
        while True:
            load, cnt, t = heapq.heappop(heap)
            if cnt >= TILE_N:
                continue  # stale/full
            if load + dv > TILE_EDGE_CAP:
                deferred.append((load, cnt, t))
                continue
            break
        tile_nodes[t].append(int(v))
        tile_load[t] = load + dv
        heapq.heappush(heap, (load + dv, cnt + 1, t))
        for item in deferred:
            heapq.heappush(heap, item)
        deferred = []
    assert max(tile_load) <= TILE_EDGE_CAP

    # sort tiles by load desc, group by 8, core c takes c-th of each group
    tsort = np.argsort(-tile_load, kind='stable')
    tile_assign = np.empty((N_CORES, TILES_PER_CORE), dtype=np.int64)
    for k in range(TILES_PER_CORE):
        for c in range(N_CORES):
            tile_assign[c, k] = tsort[k * N_CORES + c]

    # row mapping: row = c*ROWS_PER_CORE + k*128 + lane
    row_of_node = np.full(N_NODES, -1, dtype=np.int64)
    node_of_row = np.full(N_ROWS, -1, dtype=np.int64)  # -1 = virtual pad node
    for c in range(N_CORES):
        for k in range(TILES_PER_CORE):
            t = tile_assign[c, k]
            nodes = tile_nodes[t]
            base = c * ROWS_PER_CORE + k * TILE_N
            for lane, v in enumerate(nodes):
                row_of_node[v] = base + lane
                node_of_row[base + lane] = v
    assert (row_of_node >= 0).all()

    # --- per-core edge slot tables ---
    dst_row = row_of_node[dst]
    src_row = row_of_node[src]
    e_core = dst_row // ROWS_PER_CORE
    e_tile = (dst_row % ROWS_PER_CORE) // TILE_N   # k within core
    e_lane = dst_row % TILE_N

    idx_flat = np.zeros((N_CORES, SLOTS), dtype=np.int16)      # pad idx = 0
    dst_flat = np.full((N_CORES, SLOTS), -1, dtype=np.int16)   # pad dst = -1

    # gather-row renumbering: full tables are laid out chunk-major
    # ([all cores' seg-0 rows, then seg-1, ...]) so each chunked AllGather
    # output is contiguous
    seg_start = np.asarray(SEG_START)
    seg_r = np.asarray(SEG_R)
    seg_full = np.asarray(SEG_FULL)
    sc = src_row // ROWS_PER_CORE
    sr = src_row % ROWS_PER_CORE
    seg_i = np.searchsorted(seg_start, sr, side='right') - 1
    src_grow = seg_full[seg_i] + sc * seg_r[seg_i] + (sr - seg_start[seg_i])

    # group edges by (core, tile); within each tile sort by src row (gather
    # locality), then assign slot positions skipping reserved slots 1023/2047
    key = e_core * TILES_PER_CORE + e_tile
    eorder = np.lexsort((src_grow, key))   # sort by key, then src gather-row
    key_s = key[eorder]
    grp_start = np.searchsorted(key_s, np.arange(N_CORES * TILES_PER_CORE))
    pos_in_grp = np.arange(n_edges) - grp_start[key_s]
    j = pos_in_grp
    slot_in_tile = j + (j >= 1023).astype(np.int64)  # j>=1023 shifts past slot 1023
    assert slot_in_tile.max() < SLOTS_PER_TILE - 1   # never hits 2047
    slots_abs = key_s % TILES_PER_CORE * SLOTS_PER_TILE + slot_in_tile
    cores_s = key_s // TILES_PER_CORE
    idx_flat[cores_s, slots_abs] = (src_grow[eorder] - IDX_OFF).astype(np.int16)
    dst_flat[cores_s, slots_abs] = e_lane[eorder].astype(np.int16)

    # wrap idx to [128, SLOTS/16] (idx i -> [i%16 replicated, i//16])
    cols = SLOTS // 16
    idx_tile = np.zeros((N_CORES, 128, cols), dtype=np.int16)
    for c in range(N_CORES):
        w = idx_flat[c].reshape(cols, 16).T  # [16, cols]
        idx_tile[c] = np.tile(w, (8, 1))

    # dst lane per slot, wrapped [128 lanes, CHUNKS] bf16 (for on-chip sel-gen)
    dstv = np.empty((N_CORES, 128, CHUNKS), dtype=BF16)
    for c in range(N_CORES):
        dstv[c] = dst_flat[c].reshape(CHUNKS, 128).T.astype(BF16)

    # per-core normalization tables
    cd_row = np.where(node_of_row >= 0, c_dst[np.maximum(node_of_row, 0)], 1.0)
    cs_row = np.where(node_of_row >= 0, c_src[np.maximum(node_of_row, 0)], 1.0)
    cd_core = cd_row.reshape(N_CORES, ROWS_PER_CORE).astype(np.float32)
    cs_core = cs_row.reshape(N_CORES, ROWS_PER_CORE).astype(np.float32)
    cdst_pp = cd_core.reshape(N_CORES, TILES_PER_CORE, 128).transpose(0, 2, 1).copy()
    csrc_t = cs_core.reshape(N_CORES, TILES_PER_CORE, 128).transpose(0, 2, 1).copy()

    return dict(row_of_node=row_of_node, node_of_row=node_of_row,
                idx_tile=idx_tile, dstv=dstv,
                cdst_pp=cdst_pp, csrc_t=csrc_t)


def _build_nc():
    nc = bacc.Bacc("TRN2", target_bir_lowering=False, debug=False,
                   enable_asserts=True, num_devices=N_CORES, num_swdge_queues=4)
    dt = mybir.dt
    inp = {}
    inp['xT'] = nc.dram_tensor("xT", [F_IN_P, ROWS_PER_CORE], dt.bfloat16, kind="ExternalInput")
    inp['W1'] = nc.dram_tensor("W1", [F_IN_P, F1], dt.bfloat16, kind="ExternalInput")
    inp['W2'] = nc.dram_tensor("W2", [F1, F2], dt.bfloat16, kind="ExternalInput")
    inp['W3'] = nc.dram_tensor("W3", [F2, F3], dt.bfloat16, kind="ExternalInput")
    inp['idx'] = nc.dram_tensor("idx", [128, SLOTS // 16], dt.int16, kind="ExternalInput")
    inp['dstv'] = nc.dram_tensor("dstv", [128, CHUNKS], dt.bfloat16, kind="ExternalInput")
    inp['iota'] = nc.dram_tensor("iota", [128, CPC, 128], dt.bfloat16, kind="ExternalInput")
    inp['ident'] = nc.dram_tensor("ident", [128, 128], dt.bfloat16, kind="ExternalInput")
    inp['cdst_pp'] = nc.dram_tensor("cdst_pp", [128, TILES_PER_CORE], dt.float32, kind="ExternalInput")
    inp['csrc_t'] = nc.dram_tensor("csrc_t", [128, TILES_PER_CORE], dt.float32, kind="ExternalInput")
    inp['b1r'] = nc.dram_tensor("b1r", [128, F1], dt.float32, kind="ExternalInput")
    inp['b2r'] = nc.dram_tensor("b2r", [128, F2], dt.float32, kind="ExternalInput")
    inp['b3r'] = nc.dram_tensor("b3r", [128, F3], dt.float32, kind="ExternalInput")
    out_t = nc.dram_tensor("out", [ROWS_PER_CORE, F3], dt.float32, kind="ExternalOutput")

    m_own = {}
    for li, w in ((1, F1), (2, FPAD), (3, FPAD)):
        for sg in range(4):
            m_own[li, sg] = nc.dram_tensor(f"m{li}_own_{sg}", [SEG_R[sg], w], dt.bfloat16)
    m_full = {
        1: nc.dram_tensor("m1_full", [N_ROWS, F1], dt.bfloat16, addr_space="Shared"),
        2: nc.dram_tensor("m2_full", [N_ROWS, FPAD], dt.bfloat16, addr_space="Shared"),
        3: nc.dram_tensor("m3_full", [N_ROWS, FPAD], dt.bfloat16, addr_space="Shared"),
    }

    AL = mybir.AluOpType
    AF = mybir.ActivationFunctionType
    RG = [list(range(N_CORES))]

    def ag_seg(li, sg):
        """AllGather segment sg of layer li's table (contiguous chunk-major rows)."""
        lo = SEG_FULL[sg]
        hi = lo + N_CORES * SEG_R[sg]
        nc.gpsimd.collective_compute(
            "AllGather", AL.bypass, replica_groups=RG,
            ins=[m_own[li, sg][:, :]], outs=[m_full[li][lo:hi, :]])

    def m_store(li, t_idx, ob, w):
        sg = int(np.searchsorted(np.asarray(SEG_START), t_idx * 128, side='right')) - 1
        r = t_idx * 128 - SEG_START[sg]
        nc.sync.dma_start(m_own[li, sg][r:r + 128, 0:w], ob[:])
        if t_idx in AG_FIRE:
            ag_seg(li, AG_FIRE[t_idx])

    with tile.TileContext(nc) as tc:
        with tc.tile_pool(name="const", bufs=1) as constp, \
             tc.tile_pool(name="big", bufs=1) as bigp, \
             tc.tile_pool(name="xstream", bufs=2) as xp, \
             tc.tile_pool(name="work", bufs=4) as wp, \
             tc.tile_pool(name="gpool", bufs=8) as gp, \
             tc.tile_pool(name="selp", bufs=8) as selp, \
             tc.tile_pool(name="psA", bufs=2, space="PSUM") as psA, \
             tc.tile_pool(name="psT", bufs=2, space="PSUM") as psT, \
             tc.tile_pool(name="psmm", bufs=2, space="PSUM") as psmm:

            # ---- resident constants ----
            w1_t = constp.tile([128, KC1, F1], mybir.dt.bfloat16)
            nc.sync.dma_start(w1_t[:], inp['W1'].rearrange("(kc p) n -> p kc n", p=128))
            w2_t = constp.tile([128, 2, F2], mybir.dt.bfloat16)
            nc.sync.dma_start(w2_t[:], inp['W2'].rearrange("(kc p) n -> p kc n", p=128))
            w3_t = constp.tile([F2, F3], mybir.dt.bfloat16)
            nc.sync.dma_start(w3_t[:], inp['W3'][:, :])
            idx_t = constp.tile([128, SLOTS // 16], mybir.dt.int16)
            nc.sync.dma_start(idx_t[:], inp['idx'][:, :])
            dstv_t = constp.tile([128, CHUNKS], mybir.dt.bfloat16)
            nc.sync.dma_start(dstv_t[:], inp['dstv'][:, :])
            iota_t = constp.tile([128, CPC, 128], mybir.dt.bfloat16)
            nc.sync.dma_start(iota_t[:], inp['iota'][:, :, :])
            ident_t = constp.tile([128, 128], mybir.dt.bfloat16)
            nc.sync.dma_start(ident_t[:], inp['ident'][:, :])
            cdpp_t = constp.tile([128, TILES_PER_CORE], mybir.dt.float32)
            nc.sync.dma_start(cdpp_t[:], inp['cdst_pp'][:, :])
            cs_t = constp.tile([128, TILES_PER_CORE], mybir.dt.float32)
            nc.sync.dma_start(cs_t[:], inp['csrc_t'][:, :])
            b1r_t = constp.tile([128, F1], mybir.dt.float32)
            nc.sync.dma_start(b1r_t[:], inp['b1r'][:, :])
            b2r_t = constp.tile([128, F2], mybir.dt.float32)
            nc.sync.dma_start(b2r_t[:], inp['b2r'][:, :])
            b3r_t = constp.tile([128, F3], mybir.dt.float32)
            nc.sync.dma_start(b3r_t[:], inp['b3r'][:, :])

            h1t = bigp.tile([128, 2, ROWS_PER_CORE], mybir.dt.bfloat16)  # H1.T
            h2t = bigp.tile([F2, ROWS_PER_CORE], mybir.dt.bfloat16)      # H2.T

            # ---- phase 1: M1 = (X @ W1) * c_src, row-chunked AllGather ----
            blocks = [(i * 512, 512) for i in range(12)] + [(6144, 128)]
            for c0, bs in blocks:
                xt = xp.tile([128, KC1, bs], mybir.dt.bfloat16, tag="xt")
                nc.sync.dma_start(
                    xt[:, :, :bs],
                    inp['xT'][:, c0:c0 + bs].rearrange("(kc p) n -> p kc n", p=128))
                for sub in range(bs // 128):
                    t_idx = (c0 + sub * 128) // 128
                    ps = psmm.tile([128, F1], mybir.dt.float32, tag="mm1")
                    for kc in range(KC1):
                        nc.tensor.matmul(ps[:], xt[:, kc, sub * 128:(sub + 1) * 128],
                                         w1_t[:, kc, :], start=(kc == 0), stop=(kc == KC1 - 1))
                    ob = wp.tile([128, F1], mybir.dt.bfloat16, tag="m1o")
                    nc.vector.tensor_scalar(ob[:], ps[:], cs_t[:, t_idx:t_idx + 1], None, AL.mult)
                    m_store(1, t_idx, ob, F1)

            # ---- agg helper: gather + on-chip sel + sel-stationary matmul ----
            def agg_layer(li, elem, fwidth, finish_tile):
                cur = {}
                for call in range(N_CALLS):
                    g = gp.tile([128, CPC, elem], mybir.dt.bfloat16, tag=f"g{elem}")
                    nc.gpsimd.dma_gather(
                        g[:], m_full[li][IDX_OFF:, :],
                        idx_t[:, call * (CALL // 16):(call + 1) * (CALL // 16)],
                        CALL, CALL, elem, queue_num=call % 4)
                    selg = selp.tile([128, CPC, 128], mybir.dt.bfloat16, tag="selg")
                    ch0 = call * CPC
                    nc.vector.tensor_tensor(
                        selg[:], iota_t[:],
                        dstv_t[:, ch0:ch0 + CPC].unsqueeze(2).broadcast_to([128, CPC, 128]),
                        AL.is_equal)
                    for j in range(CPC):
                        ch = ch0 + j
                        t_idx = ch // R_CHUNKS
                        first = (ch % R_CHUNKS == 0)
                        last = (ch % R_CHUNKS == R_CHUNKS - 1)
                        if first:
                            cur[0] = psA.tile([128, fwidth], mybir.dt.float32, tag="aggA", name="psagg")
                        nc.tensor.matmul(cur[0][:], selg[:, j, :], g[:, j, 0:fwidth],
                                         start=first, stop=last)
                        if last:
                            finish_tile(cur[0], t_idx)
                            cur.clear()

            # ---- layer 1 agg -> H1T; fused L2 dense + chunked AG2 ----
            def l1_tile(ps_agg, t_idx):
                sl = slice(t_idx * 128, (t_idx + 1) * 128)
                h1d = wp.tile([128, F1], mybir.dt.bfloat16, tag="h1d")
                nc.vector.scalar_tensor_tensor(
                    h1d[:], ps_agg[:], cdpp_t[:, t_idx:t_idx + 1], b1r_t[:],
                    AL.mult, AL.add)
                nc.scalar.activation(h1d[:], h1d[:], AF.Relu)
                for fc in range(2):
                    trp = psT.tile([128, 128], mybir.dt.bfloat16, tag="tr")
                    nc.tensor.transpose(trp[:], h1d[:, fc * 128:(fc + 1) * 128], ident_t[:])
                    nc.scalar.activation(h1t[:, fc, sl], trp[:], AF.Copy)
                # fused L2 dense for this tile
                ps2 = psmm.tile([128, F1], mybir.dt.float32, tag="mm1", name="ps2")
                for fc in range(2):
                    nc.tensor.matmul(ps2[:, 0:F2], h1t[:, fc, sl], w2_t[:, fc, :],
                                     start=(fc == 0), stop=(fc == 1))
                ob2 = wp.tile([128, F2], mybir.dt.bfloat16, tag="ob2")
                nc.vector.tensor_scalar(ob2[:], ps2[:, 0:F2], cs_t[:, t_idx:t_idx + 1], None, AL.mult)
                m_store(2, t_idx, ob2, F2)

            agg_layer(1, F1, F1, l1_tile)

            # ---- layer 2 agg -> H2T; fused L3 dense + chunked AG3 ----
            def l2_tile(ps_agg, t_idx):
                sl = slice(t_idx * 128, (t_idx + 1) * 128)
                h2d = wp.tile([128, F2], mybir.dt.bfloat16, tag="h2d")
                nc.vector.scalar_tensor_tensor(
                    h2d[:], ps_agg[:], cdpp_t[:, t_idx:t_idx + 1], b2r_t[:],
                    AL.mult, AL.add)
                nc.scalar.activation(h2d[:], h2d[:], AF.Relu)
                trp = psT.tile([128, 128], mybir.dt.bfloat16, tag="tr", name="trp2")
                nc.tensor.transpose(trp[0:F2, :], h2d[:], ident_t[:])
                nc.scalar.activation(h2t[:, sl], trp[0:F2, :], AF.Copy)
                # fused L3 dense for this tile
                ps3 = psmm.tile([128, F1], mybir.dt.float32, tag="mm1", name="ps3")
                nc.tensor.matmul(ps3[:, 0:F3], h2t[:, sl], w3_t[:], start=True, stop=True)
                ob3 = wp.tile([128, F3], mybir.dt.bfloat16, tag="ob3")
                nc.vector.tensor_scalar(ob3[:], ps3[:, 0:F3], cs_t[:, t_idx:t_idx + 1], None, AL.mult)
                m_store(3, t_idx, ob3, F3)

            agg_layer(2, FPAD, F2, l2_tile)

            # ---- layer 3 agg -> logits; softmax batched in two halves ----
            xall = bigp.tile([128, TILES_PER_CORE * F3], mybir.dt.float32)

            def softmax_part(tlo, thi):
                flo, fhi = tlo * F3, thi * F3
                exa = wp.tile([128, (thi - tlo) * F3], mybir.dt.float32,
                              tag="exa", name="exa")
                nc.scalar.activation(exa[:], xall[:, flo:fhi], AF.Exp)
                smv = wp.tile([128, thi - tlo], mybir.dt.float32, tag="smv", name="smv")
                nc.vector.tensor_reduce(
                    smv[:], exa[:].rearrange("p (t f) -> p t f", f=F3),
                    mybir.AxisListType.X, AL.add)
                rsv = wp.tile([128, thi - tlo], mybir.dt.float32, tag="rsv", name="rsv")
                nc.vector.reciprocal(rsv[:], smv[:])
                nlog = wp.tile([128, thi - tlo], mybir.dt.float32, tag="nlog", name="nlog")
                nc.scalar.activation(nlog[:], rsv[:], AF.Ln)
                for t_idx in range(tlo, thi):
                    sl3 = slice(t_idx * F3, (t_idx + 1) * F3)
                    ox = wp.tile([128, F3], mybir.dt.float32, tag="ox")
                    nc.vector.tensor_scalar(ox[:], xall[:, sl3],
                                            nlog[:, t_idx - tlo:t_idx - tlo + 1], None, AL.add)
                    nc.sync.dma_start(out_t[t_idx * 128:(t_idx + 1) * 128, :], ox[:])

            def l3_tile(ps_agg, t_idx):
                sl3 = slice(t_idx * F3, (t_idx + 1) * F3)
                nc.vector.scalar_tensor_tensor(
                    xall[:, sl3], ps_agg[:], cdpp_t[:, t_idx:t_idx + 1], b3r_t[:],
                    AL.mult, AL.add)
                if t_idx == 23:
                    softmax_part(0, 24)

            agg_layer(3, FPAD, F3, l3_tile)
            softmax_part(24, TILES_PER_CORE)

    nc.compile()
    return nc


def _install_profile_shim():
    """Provide the missing antenv.axon_hooks module so trace=True works under axon."""
    try:
        import types
        import antenv
        if 'antenv.axon_hooks' in sys.modules:
            return
        _hook = [None]
        mod = types.ModuleType('antenv.axon_hooks')
        mod.set_axon_ntff_profile_hook = lambda h: _hook.__setitem__(0, h)
        mod.get_axon_ntff_profile_hook = lambda: _hook[0]
        sys.modules['antenv.axon_hooks'] = mod
        antenv.axon_hooks = mod
        from trn_agent_boot.trn_boot import _ntff_profile_via_ctypes
        mod.set_axon_ntff_profile_hook(
            _ntff_profile_via_ctypes('/opt/axon/libaxon_pjrt.so'))
    except Exception:
        pass


_CACHE = {}


def kernel(features, edge_index, W1, b1, W2, b2, W3, b3):
    global last_exec_time_ns
    features = np.asarray(features, dtype=np.float32)
    pre = _preprocess(np.asarray(edge_index))

    if 'nc' not in _CACHE:
        _CACHE['nc'] = _build_nc()
    nc = _CACHE['nc']

    # host-side input prep
    W1p = np.zeros((F_IN_P, F1), dtype=BF16)
    W1p[:F_IN] = np.asarray(W1, dtype=BF16)
    W2b = np.asarray(W2, dtype=BF16)
    W3b = np.asarray(W3, dtype=BF16)
    b1r = np.tile(np.asarray(b1, dtype=np.float32), (128, 1))
    b2r = np.tile(np.asarray(b2, dtype=np.float32), (128, 1))
    b3r = np.tile(np.asarray(b3, dtype=np.float32), (128, 1))
    iota = np.ascontiguousarray(np.broadcast_to(
        np.arange(128, dtype=np.float32), (128, CPC, 128))).astype(BF16)
    ident = np.eye(128, dtype=BF16)

    # features, permuted and transposed per core: [F_IN_P, 6272] bf16
    feat_b = features.astype(BF16)
    in_maps = []
    for c in range(N_CORES):
        rows = pre['node_of_row'][c * ROWS_PER_CORE:(c + 1) * ROWS_PER_CORE]
        xTc = np.zeros((F_IN_P, ROWS_PER_CORE), dtype=BF16)
        real = rows >= 0
        xTc[:F_IN, real] = feat_b[rows[real]].T
        in_maps.append({
            'xT': xTc, 'W1': W1p, 'W2': W2b, 'W3': W3b,
            'idx': pre['idx_tile'][c], 'dstv': pre['dstv'][c],
            'iota': iota, 'ident': ident,
            'cdst_pp': pre['cdst_pp'][c], 'csrc_t': pre['csrc_t'][c],
            'b1r': b1r, 'b2r': b2r, 'b3r': b3r,
        })

    trace = os.environ.get('BASS_KERNEL_TRACE', '0') == '1'
    if trace:
        _install_profile_shim()
    res = run_bass_kernel_spmd(nc, in_maps, core_ids=list(range(N_CORES)), trace=trace)
    last_exec_time_ns = res.exec_time_ns

    # assemble + inverse permute
    out_rows = np.concatenate([res.results[c]['out'] for c in range(N_CORES)], axis=0)
    out = np.empty((N_NODES, F3), dtype=np.float32)
    real = pre['node_of_row'] >= 0
    out[pre['node_of_row'][real]] = out_rows[real]
    return out
